# revision 11
# baseline (speedup 1.0000x reference)
"""BillehColumn single-step kernel on 8 Trainium2 NeuronCores.

Strategy (row/neuron partition, per sharding hint):
- NC m owns neurons [m*6250, (m+1)*6250) == synapse rows [m*62500, (m+1)*62500).
- Each NC reads the full delayed spike buffer z_buf (own HBM copy; no
  collectives).
- Sparse i_rec = W @ z on-device per NC via GpSimd ap_gather: z is split into
  16 subtables of 15625 (one per partition within each 16-partition Q7 core
  group); edges are bucketed per (Q7-core k, lane j = col//15625) with per-row
  slot ranges padded to C(r) = max_j c_{r,j} so the segment structure is
  identical across the 16 lanes of a core (and, via a global class profile,
  across all cores and NCs — one SPMD program).  Per slot chunk: 16 ap_gather
  calls deliver z, 16 strided-partition SBUF DMAs assemble the useful
  diagonal, a weight-stream multiply and per-class strided tensor_reduce give
  per-lane row partials; a block-diagonal ones matmul on the PE sums the 16
  lanes of each core.
- All remaining GLIF dynamics are elementwise on [128, x] tiles.
- Host (numpy) work is limited to static graph restructuring (depends only on
  rec_idx), input layout permutation, and output reassembly.
"""
import sys, os, time
sys.path.insert(0, "/opt/trn_rl_repo")
import numpy as np
from contextlib import ExitStack

import jax
from jax.sharding import Mesh, PartitionSpec
from jax.experimental.shard_map import shard_map

import concourse.bass as bass
import concourse.tile as tile
from concourse import bacc, mybir
from concourse.bass2jax import _bass_exec_p, partition_id_tensor, install_neuronx_cc_hook

# ---------------- constants ----------------
N = 50000; R = 10; D = 5; E = 10_000_000
NC_N = 8            # NeuronCores
P = 128
NPC = N // NC_N     # neurons per NC = 6250
RPC = R * NPC       # rows per NC = 62500
ZT = D * N          # z entries = 250000
TSUB = ZT // 16     # subtable = 15625
F2 = 586            # phase-2 synapse tile free size
NRQ = 16 * F2       # padded layout rows per Q7 core = 9376 (128B-mult pitch)
FN = 49             # neuron tile free size (128*49 = 6272 >= 6250)
NNP = P * FN        # padded neurons = 6272
CH = 768            # gather slot chunk
DT = 1.0

_prep_cache = {}


def _fingerprint(rec_idx):
    a = np.ascontiguousarray(rec_idx)
    return (a.shape, a.dtype.str, hash(a[::997, 0].tobytes()),
            hash(a[:997, 1].tobytes()))


def _rank_within_group(key):
    """rank of each element within its equal-key group (stable)."""
    n = len(key)
    sidx = np.argsort(key, kind="stable")
    sk = key[sidx]
    first = np.r_[True, sk[1:] != sk[:-1]]
    grp_start_idx = np.nonzero(first)[0]
    grp_id = np.cumsum(first) - 1
    rank_sorted = np.arange(n) - grp_start_idx[grp_id]
    rank = np.empty(n, np.int64)
    rank[sidx] = rank_sorted
    return rank




def _balance_lanes(r_loc, col_g, rng_seed=12345):
    """Assign each z-column to one of 16 lanes minimizing sum_r max_j c_{r,j}.
    Returns lane[E], off[E], zperm [16, TSUB] (z index per (lane,off), -1 pad)."""
    rng = np.random.default_rng(rng_seed)
    deg = np.bincount(col_g, minlength=ZT)
    order = np.argsort(-deg, kind="stable")
    es = np.argsort(col_g, kind="stable")
    row_sorted = r_loc[es]
    starts = np.searchsorted(col_g[es], np.arange(ZT + 1))
    counts = np.zeros((RPC, 16), np.int32)
    loads = np.zeros(16, np.int64)
    lane_of_col = np.full(ZT, -1, np.int32)
    B = 8192
    for b0 in range(0, ZT, B):
        batch = order[b0:b0 + B]
        batch = batch[deg[batch] > 0]
        if len(batch) == 0:
            continue
        lens = deg[batch]
        tot = lens.sum()
        eidx = np.repeat(starts[batch], lens) + (
            np.arange(tot) - np.repeat(np.cumsum(lens) - lens, lens))
        rr = row_sorted[eidx]
        cloc = np.repeat(np.arange(len(batch)), lens)
        score = np.zeros((len(batch), 16), np.float64)
        np.add.at(score, cloc, counts[rr].astype(np.float64))
        score += rng.random(score.shape) * 0.01
        score[:, loads >= TSUB] = np.inf
        lane_b = np.argmin(score, 1).astype(np.int32)
        lane_of_col[batch] = lane_b
        np.add.at(counts, (rr, lane_b[cloc]), 1)
        loads += np.bincount(lane_b, minlength=16)
    z0 = np.nonzero(lane_of_col < 0)[0]
    free = np.repeat(np.arange(16), np.maximum(TSUB - loads, 0))
    lane_of_col[z0] = free[:len(z0)]
    # fix any lane overflow: move lowest-degree cols to under-full lanes
    loads = np.bincount(lane_of_col, minlength=16)
    if loads.max() > TSUB:
        for j in np.nonzero(loads > TSUB)[0]:
            cj = np.nonzero(lane_of_col == j)[0]
            cj = cj[np.argsort(deg[cj])]  # move low-degree first
            surplus = cj[:loads[j] - TSUB]
            for c in surplus:
                tgt = int(np.argmin(loads))
                lane_of_col[c] = tgt
                loads[tgt] += 1; loads[j] -= 1
    off_of_col = np.zeros(ZT, np.int64)
    zperm = np.full((16, TSUB), -1, np.int64)
    for j in range(16):
        cj = np.nonzero(lane_of_col == j)[0]
        off_of_col[cj] = np.arange(len(cj))
        zperm[j, :len(cj)] = cj
    return (lane_of_col[col_g].astype(np.int64), off_of_col[col_g], zperm)

def _prep_static(rec_idx, rec_w):
    rows_g = rec_idx[:, 0].astype(np.int64)
    cols_g = rec_idx[:, 1].astype(np.int64)
    w_all = np.asarray(rec_w, np.float32)
    nc_id = rows_g // RPC

    # pass 1: per-NC per-edge fields + global class profile
    percore = []
    for m in range(NC_N):
        em = np.nonzero(nc_id == m)[0]
        r_loc = rows_g[em] - m * RPC
        lane, off, zperm = _balance_lanes(r_loc, cols_g[em])
        w = w_all[em]
        counts = np.bincount(r_loc * 16 + lane, minlength=RPC * 16).reshape(RPC, 16)
        Cr = np.maximum(counts.max(1).astype(np.int64), 1)
        order = np.argsort(-Cr, kind="stable")
        snake = np.tile(np.r_[np.arange(8), np.arange(7, -1, -1)],
                        RPC // 16 + 1)[:RPC]
        core_of_row = np.empty(RPC, np.int64)
        core_of_row[order] = snake
        percore.append((r_loc, lane, off, w, Cr, core_of_row, zperm))

    cmax = int(max(p[4].max() for p in percore))
    # global per-class row-count profile: max over all (NC, core) pairs
    prof = np.zeros(cmax + 1, np.int64)
    for (r_loc, lane, off, w, Cr, corow, zperm) in percore:
        for k in range(8):
            sel = corow == k
            h = np.bincount(Cr[sel], minlength=cmax + 1)
            prof = np.maximum(prof, h)
    classes = [c for c in range(cmax, 0, -1) if prof[c] > 0]

    # shared layout: position/slot for every (class, i) with CH-straddle padding
    plan = []  # (class, rowpos0, nrows, slot0) each within one chunk
    pos = 0; slot = 0
    cls_bounds = {}  # class -> (pos0, nrows)
    for c in classes:
        nr_ = int(prof[c]); done = 0
        cls_bounds[c] = (pos, nr_)
        while done < nr_:
            room = (-slot) % CH
            if room == 0:
                room = CH
            fit = min(nr_ - done, room // c)
            if fit == 0:
                slot += room
                continue
            plan.append((int(c), int(pos), int(fit), int(slot)))
            pos += fit; slot += fit * c; done += fit
    NROW_USED = pos
    NSLOT = int(-(-slot // CH) * CH)
    NCHUNK = NSLOT // CH
    assert NROW_USED <= NRQ, (NROW_USED, NRQ)
    # slot base per layout position
    slot_of_pos = np.zeros(NROW_USED, np.int64)
    for (c, p0, nr_, s0) in plan:
        slot_of_pos[p0:p0 + nr_] = s0 + np.arange(nr_) * c

    out = []
    for m in range(NC_N):
        (r_loc, lane, off, w, Cr, corow, zperm) = percore[m]
        # per-core: assign rows of class c to positions cls_bounds[c]
        rowpos = np.full((8, RPC), -1, np.int64)
        layout_rows = np.full((8, NRQ), -1, np.int64)
        for k in range(8):
            rk = np.nonzero(corow == k)[0]
            crk = Cr[rk]
            ordk = np.argsort(-crk, kind="stable")
            rk = rk[ordk]; crk = crk[ordk]
            # positions: within class c, i-th row -> cls_bounds[c][0] + i
            ranks = _rank_within_group(-crk)
            posk = np.array([cls_bounds[int(c)][0] for c in crk]) + ranks
            rowpos[k, rk] = posk
            layout_rows[k, posk] = rk

        ek = corow[r_loc]
        epos = rowpos[ek, r_loc]
        rank = _rank_within_group(r_loc * 16 + lane)
        slot_e = slot_of_pos[epos] + rank

        O = np.zeros((8, 16, NSLOT), np.int16)
        W = np.zeros((8, 16, NSLOT), np.float32)
        O[ek, lane, slot_e] = off.astype(np.int16)
        W[ek, lane, slot_e] = w

        idxs = np.zeros((NCHUNK, P, 16, CH // 16), np.int16)
        Wd = np.zeros((NCHUNK, P, CH), np.float32)
        Ov = O.reshape(8, 16, NCHUNK, CH)
        Wv = W.reshape(8, 16, NCHUNK, CH)
        for k in range(8):
            for j in range(16):
                idxs[:, 16 * k:16 * k + 16, j, :] = (
                    Ov[k, j].reshape(NCHUNK, CH // 16, 16).transpose(0, 2, 1))
                Wd[:, 16 * k + j, :] = Wv[k, j]

        rho = np.full(8 * NRQ, -1, np.int64)
        for k in range(8):
            rho[k * NRQ:(k + 1) * NRQ] = layout_rows[k]
        out.append(dict(idxs=idxs, Wd=Wd, rho=rho, zperm=zperm))
    return dict(percore=out, plan=plan, NCHUNK=NCHUNK, NSLOT=NSLOT)


# ---------------- device kernel ----------------
def _build_kernel(NCHUNK, plan):
    nc = bacc.Bacc("TRN2", target_bir_lowering=False, debug=False,
                   num_devices=NC_N)
    f32, i16 = mybir.dt.float32, mybir.dt.int16
    DI = lambda n, s, d=f32: nc.dram_tensor(n, s, d, kind="ExternalInput").ap()
    DO = lambda n, s, d=f32: nc.dram_tensor(n, s, d, kind="ExternalOutput").ap()

    z2d_d = DI("z2d", [16, TSUB])
    idxs_d = DI("idxs", [NCHUNK, P, 16, CH // 16], i16)
    wstr_d = DI("wstr", [NCHUNK, P, CH])
    ones_d = DI("onesb", [P, 8])
    syn_d = DI("syn", [5, P, F2])     # inputs, psc_rise, psc, syn_decay, psc_initial
    pscn_d = DI("pscn", [P, FN * R])  # psc in neuron order (padded)
    nrn_d = DI("nrn", [18, P, FN])
    nz_d = DO("nz", [P, FN]); outv_d = DO("outv", [P, FN])
    nv_d = DO("nv", [P, FN]); nr_d = DO("nr", [P, FN])
    na1_d = DO("na1", [P, FN]); na2_d = DO("na2", [P, FN])
    nprise_d = DO("nprise", [P, F2]); npsc_d = DO("npsc", [P, F2])

    AluOp = mybir.AluOpType
    ActF = mybir.ActivationFunctionType

    with tile.TileContext(nc) as tc, ExitStack() as octx:
        opool = octx.enter_context(tc.tile_pool(name="outer", bufs=1))
        partial = opool.tile([P, NRQ], f32)
        nc.vector.memset(partial[:], 0.0)

        with ExitStack() as gctx:
            gpool = gctx.enter_context(tc.tile_pool(name="g", bufs=1))
            dbl = gctx.enter_context(tc.tile_pool(name="gdbl", bufs=2))
            ztab = gpool.tile([P, TSUB], f32)
            for k in range(8):
                nc.sync.dma_start(ztab[16 * k:16 * (k + 1), :], z2d_d[:])
            T16 = gpool.tile([P, 16, CH], f32)
            for c in range(NCHUNK):
                idxt = dbl.tile([P, 16 * (CH // 16)], i16, tag="idx")
                nc.sync.dma_start(idxt[:],
                                  idxs_d[c].rearrange("p j s -> p (j s)"))
                wt = dbl.tile([P, CH], f32, tag="w")
                nc.sync.dma_start(wt[:], wstr_d[c])
                for j in range(16):
                    nc.gpsimd.ap_gather(
                        out_ap=T16[:, j, :], in_ap=ztab[:],
                        idxs_ap=idxt[:, j * (CH // 16):(j + 1) * (CH // 16)],
                        channels=P, num_elems=TSUB, d=1, num_idxs=CH)
                V = dbl.tile([P, CH], f32, tag="v")
                for j in range(16):
                    nc.sync.dma_start(V[j:P:16, :], T16[j:P:16, j, :])
                nc.vector.tensor_mul(V[:], V[:], wt[:])
                for (cc, rowpos0, nrows, slot0) in plan:
                    if slot0 // CH != c:
                        continue
                    s0 = slot0 - c * CH
                    if cc == 1:
                        nc.vector.tensor_copy(
                            partial[:, rowpos0:rowpos0 + nrows],
                            V[:, s0:s0 + nrows])
                    else:
                        nc.vector.tensor_reduce(
                            partial[:, rowpos0:rowpos0 + nrows],
                            V[:, s0:s0 + nrows * cc].rearrange(
                                "p (n c) -> p n c", c=cc),
                            axis=mybir.AxisListType.X, op=AluOp.add)

        with ExitStack() as mctx:
            mpool = mctx.enter_context(tc.tile_pool(name="m", bufs=1))
            psum = mctx.enter_context(
                tc.tile_pool(name="ps", bufs=2, space="PSUM"))
            onesb = mpool.tile([P, 8], f32)
            nc.sync.dma_start(onesb[:], ones_d[:])
            irec8 = mpool.tile([8, NRQ], f32)
            t0 = 0
            while t0 < NRQ:
                csz = min(512, NRQ - t0)
                pt = psum.tile([8, 512], f32, space="PSUM", tag="pt")
                nc.tensor.matmul(out=pt[:, :csz], lhsT=onesb[:],
                                 rhs=partial[:, t0:t0 + csz],
                                 start=True, stop=True)
                nc.vector.tensor_copy(irec8[:, t0:t0 + csz], pt[:, :csz])
                t0 += csz
            # reshape [8, NRQ] -> [128, F2]: partition 16k+t <- irec8[k, t*F2:..]
            from concourse.ap import AP as RawAP
            i8ap = irec8[:]
            pitch = i8ap.ap[0][0]
            irec2 = mpool.tile([P, F2], f32)
            src = RawAP(tensor=i8ap.tensor, offset=i8ap.offset,
                        ap=[[pitch, 8], [F2, 16], [1, F2]])
            nc.sync.dma_start(irec2[:], src)

            # ---------------- phase 2 ----------------
            syn = [mpool.tile([P, F2], f32, tag=f"syn{i}", name=f"syn{i}") for i in range(5)]
            for i in range(5):
                nc.sync.dma_start(syn[i][:], syn_d[i])
            inp, prise, psc, sdec, pinit = syn
            pscn = mpool.tile([P, FN * R], f32)
            nc.sync.dma_start(pscn[:], pscn_d[:])
            nrn = [mpool.tile([P, FN], f32, tag=f"nrn{i}", name=f"nrn{i}") for i in range(18)]
            for i in range(18):
                nc.sync.dma_start(nrn[i][:], nrn_d[i])
            (v, r_, asc1, asc2, vth, el, vrst, g, dec, cf, tref,
             k0, k1, amp0, amp1, vscl, voff, pz) = nrn

            t1 = mpool.tile([P, F2], f32); t2 = mpool.tile([P, F2], f32)
            nc.vector.tensor_add(t1[:], irec2[:], inp[:])
            nc.vector.tensor_mul(t1[:], t1[:], pinit[:])
            nc.vector.tensor_mul(t2[:], sdec[:], prise[:])
            nc.vector.tensor_add(t1[:], t1[:], t2[:])
            nc.sync.dma_start(nprise_d[:], t1[:])
            nc.vector.tensor_add(t2[:], psc[:], prise[:])
            nc.vector.tensor_mul(t2[:], t2[:], sdec[:])
            nc.sync.dma_start(npsc_d[:], t2[:])

            a = mpool.tile([P, FN], f32); b = mpool.tile([P, FN], f32)
            c_ = mpool.tile([P, FN], f32); d_ = mpool.tile([P, FN], f32)
            cur = mpool.tile([P, FN], f32)
            nc.vector.tensor_reduce(
                cur[:], pscn[:].rearrange("p (n r) -> p n r", r=R),
                axis=mybir.AxisListType.X, op=AluOp.add)
            for (kk, ascx, amp, outd) in ((k0, asc1, amp0, na1_d),
                                          (k1, asc2, amp1, na2_d)):
                nc.scalar.activation(a[:], kk[:], ActF.Sigmoid)
                nc.scalar.activation(a[:], a[:], ActF.Exp, scale=-1.0)
                nc.vector.tensor_mul(a[:], a[:], ascx[:])
                nc.vector.tensor_mul(b[:], pz[:], amp[:])
                nc.vector.tensor_add(a[:], a[:], b[:])
                nc.sync.dma_start(outd[:], a[:])
            nc.vector.tensor_mul(a[:], pz[:], tref[:])
            nc.vector.tensor_add(a[:], a[:], r_[:])
            nc.vector.tensor_scalar_add(a[:], a[:], -DT)
            nc.vector.tensor_scalar_max(a[:], a[:], 0.0)
            nc.sync.dma_start(nr_d[:], a[:])          # a = new_r
            nc.vector.tensor_mul(b[:], g[:], el[:])
            nc.vector.tensor_add(b[:], b[:], cur[:])
            nc.vector.tensor_add(b[:], b[:], asc1[:])
            nc.vector.tensor_add(b[:], b[:], asc2[:])
            nc.vector.tensor_mul(b[:], b[:], cf[:])
            nc.vector.tensor_mul(c_[:], dec[:], v[:])
            nc.vector.tensor_add(b[:], b[:], c_[:])   # b = new_v (pre-reset)
            nc.vector.tensor_sub(c_[:], vth[:], el[:])
            nc.vector.reciprocal(c_[:], c_[:])
            nc.vector.tensor_sub(d_[:], b[:], vth[:])
            nc.vector.tensor_mul(d_[:], d_[:], c_[:])
            nc.vector.tensor_scalar(d_[:], d_[:], 0.0, None, op0=AluOp.is_gt)
            nc.vector.tensor_scalar(c_[:], a[:], 0.0, None, op0=AluOp.is_le)
            nc.vector.tensor_mul(d_[:], d_[:], c_[:])  # d_ = new_z
            nc.sync.dma_start(nz_d[:], d_[:])
            # blend: new_v = vrst*nz + b*(1-nz)
            nc.vector.tensor_scalar(t1[:, :FN], d_[:], -1.0, 1.0,
                                    op0=AluOp.mult, op1=AluOp.add)
            nc.vector.tensor_mul(b[:], b[:], t1[:, :FN])
            nc.vector.tensor_mul(c_[:], vrst[:], d_[:])
            nc.vector.tensor_add(c_[:], c_[:], b[:])
            nc.sync.dma_start(nv_d[:], c_[:])
            nc.vector.tensor_mul(c_[:], c_[:], vscl[:])
            nc.vector.tensor_add(c_[:], c_[:], voff[:])
            nc.sync.dma_start(outv_d[:], c_[:])
    nc.compile()
    return nc


# ---------------- runner ----------------
class _Runner:
    def __init__(self, nc, n_cores=NC_N):
        install_neuronx_cc_hook()
        self.nc = nc; self.n_cores = n_cores
        in_names, out_names, out_avals = [], [], []
        pname = nc.partition_id_tensor.name if nc.partition_id_tensor else None
        for alloc in nc.m.functions[0].allocations:
            if not isinstance(alloc, mybir.MemoryLocationSet):
                continue
            name = alloc.memorylocations[0].name
            if alloc.kind == "ExternalInput":
                if name != pname:
                    in_names.append(name)
            elif alloc.kind == "ExternalOutput":
                out_names.append(name)
                out_avals.append(jax.core.ShapedArray(
                    tuple(alloc.tensor_shape), mybir.dt.np(alloc.dtype)))
        self.in_names, self.out_names, self.out_avals = in_names, out_names, out_avals
        n_params = len(in_names); n_outs = len(out_avals)
        all_in = in_names + out_names + ([pname] if pname else [])
        donate = tuple(range(n_params, n_params + n_outs))
        use_pid = pname is not None

        def _body(*args):
            operands = list(args)
            if use_pid:
                operands.append(partition_id_tensor())
            return tuple(_bass_exec_p.bind(
                *operands, out_avals=tuple(out_avals), in_names=tuple(all_in),
                out_names=tuple(out_names), lowering_input_output_aliases=(),
                sim_require_finite=False, sim_require_nnan=False, nc=nc))

        devices = jax.devices()[:n_cores]
        mesh = Mesh(np.asarray(devices), ("core",))
        self.fn = jax.jit(
            shard_map(_body, mesh=mesh,
                      in_specs=(PartitionSpec("core"),) * (n_params + n_outs),
                      out_specs=(PartitionSpec("core"),) * n_outs,
                      check_rep=False),
            donate_argnums=donate, keep_unused=True)
        self.n_params = n_params

    def run(self, in_maps):
        per_core = [[np.asarray(m[n]) for n in self.in_names] for m in in_maps]
        cat = [np.concatenate([per_core[c][i] for c in range(self.n_cores)], axis=0)
               for i in range(self.n_params)]
        zeros = [np.zeros((self.n_cores * a.shape[0], *a.shape[1:]), a.dtype)
                 for a in self.out_avals]
        outs = self.fn(*cat, *zeros)
        jax.block_until_ready(outs)
        return [
            {n: np.asarray(outs[i]).reshape(self.n_cores,
                                            *self.out_avals[i].shape)[c]
             for i, n in enumerate(self.out_names)}
            for c in range(self.n_cores)
        ]


_kernel_cache = {}


def kernel(inputs, z_buf, v, r, asc_1, asc_2, psc_rise, psc, rec_w, rec_idx,
           v_th, e_l, v_reset, g, decay, current_factor, t_ref, k, asc_amps,
           syn_decay, psc_initial, voltage_scale, voltage_offset):
    fp = _fingerprint(np.asarray(rec_idx))
    if fp not in _prep_cache:
        _prep_cache[fp] = _prep_static(np.asarray(rec_idx), np.asarray(rec_w))
    prep = _prep_cache[fp]
    NCHUNK, plan = prep["NCHUNK"], prep["plan"]

    kkey = (NCHUNK, tuple(plan))
    if kkey not in _kernel_cache:
        _kernel_cache[kkey] = _Runner(_build_kernel(NCHUNK, plan))
    runner = _kernel_cache[kkey]

    onesb = np.zeros((P, 8), np.float32)
    for pp in range(P):
        onesb[pp, pp // 16] = 1.0
    z_full = np.asarray(z_buf, np.float32).reshape(ZT)

    in_maps = []
    for m in range(NC_N):
        pr = prep["percore"][m]
        zperm = pr["zperm"]
        z2d = np.where(zperm >= 0, z_full[np.maximum(zperm, 0)], 0.0
                       ).astype(np.float32)
        rho = pr["rho"]
        valid = rho >= 0
        gsl = slice(m * RPC, (m + 1) * RPC)
        nsl = slice(m * NPC, (m + 1) * NPC)

        def synp(x):
            x = np.asarray(x, np.float32).reshape(-1)[gsl]
            out = np.zeros(8 * NRQ, np.float32)
            out[valid] = x[rho[valid]]
            return out.reshape(P, F2)

        syn = np.stack([synp(inputs), synp(psc_rise), synp(psc),
                        synp(syn_decay), synp(psc_initial)])
        pscn = np.zeros(NNP * R, np.float32)
        pscn[:RPC] = np.asarray(psc, np.float32).reshape(-1)[gsl]
        nfield = lambda x: np.pad(np.asarray(x, np.float32).reshape(-1)[nsl],
                                  (0, NNP - NPC)).reshape(P, FN)
        k_ = np.asarray(k, np.float32); aa = np.asarray(asc_amps, np.float32)
        nrn = np.stack([
            nfield(v), nfield(r), nfield(asc_1), nfield(asc_2), nfield(v_th),
            nfield(e_l), nfield(v_reset), nfield(g), nfield(decay),
            nfield(current_factor), nfield(t_ref), nfield(k_[:, 0]),
            nfield(k_[:, 1]), nfield(aa[:, 0]), nfield(aa[:, 1]),
            nfield(voltage_scale), nfield(voltage_offset),
            nfield(z_full[:N]),
        ])
        in_maps.append(dict(z2d=z2d, idxs=pr["idxs"], wstr=pr["Wd"],
                            onesb=onesb, syn=syn,
                            pscn=pscn.reshape(P, FN * R), nrn=nrn))

    global _last
    _last = (runner, in_maps)
    res = runner.run(in_maps)

    new_z = np.zeros(N, np.float32); out_v = np.zeros(N, np.float32)
    new_v = np.zeros(N, np.float32); new_r = np.zeros(N, np.float32)
    na1 = np.zeros(N, np.float32); na2 = np.zeros(N, np.float32)
    nprise = np.zeros(R * N, np.float32); npsc = np.zeros(R * N, np.float32)
    for m in range(NC_N):
        o = res[m]
        nsl = slice(m * NPC, (m + 1) * NPC)
        gsl = slice(m * RPC, (m + 1) * RPC)
        new_z[nsl] = o["nz"].reshape(-1)[:NPC]
        out_v[nsl] = o["outv"].reshape(-1)[:NPC]
        new_v[nsl] = o["nv"].reshape(-1)[:NPC]
        new_r[nsl] = o["nr"].reshape(-1)[:NPC]
        na1[nsl] = o["na1"].reshape(-1)[:NPC]
        na2[nsl] = o["na2"].reshape(-1)[:NPC]
        rho = prep["percore"][m]["rho"]; valid = rho >= 0
        t = np.zeros(RPC, np.float32)
        t[rho[valid]] = o["nprise"].reshape(-1)[valid]
        nprise[gsl] = t
        t = np.zeros(RPC, np.float32)
        t[rho[valid]] = o["npsc"].reshape(-1)[valid]
        npsc[gsl] = t

    zb = np.asarray(z_buf, np.float32).reshape(1, ZT)
    new_z_buf = np.concatenate([new_z.reshape(1, N), zb[:, :(D - 1) * N]],
                               axis=1)
    return (new_z.reshape(1, N), out_v.reshape(1, N), new_z_buf,
            new_v.reshape(1, N), new_r.reshape(1, N), na1.reshape(1, N),
            na2.reshape(1, N), nprise.reshape(1, R * N), npsc.reshape(1, R * N))


_last = None


def hw_time_s(iters=5):
    """Device-execution wall estimate: jitted fn with device-resident inputs."""
    runner, in_maps = _last
    per_core = [[np.asarray(m[n]) for n in runner.in_names] for m in in_maps]
    cat = [np.concatenate([per_core[c][i] for c in range(runner.n_cores)], axis=0)
           for i in range(runner.n_params)]
    ins = [jax.device_put(x) for x in cat]
    jax.block_until_ready(ins)
    zeros = lambda: [np.zeros((runner.n_cores * a.shape[0], *a.shape[1:]), a.dtype)
                     for a in runner.out_avals]
    o = runner.fn(*ins, *zeros()); jax.block_until_ready(o)
    import time as _t
    ts = []
    for _ in range(iters):
        t0 = _t.perf_counter()
        o = runner.fn(*ins, *zeros()); jax.block_until_ready(o)
        ts.append(_t.perf_counter() - t0)
    return min(ts)


# revision 12
# speedup vs baseline: 1.0098x; 1.0098x over previous
"""BillehColumn single-step kernel on 8 Trainium2 NeuronCores.

Strategy (row/neuron partition, per sharding hint):
- NC m owns neurons [m*6250, (m+1)*6250) == synapse rows [m*62500, (m+1)*62500).
- Each NC reads the full delayed spike buffer z_buf (own HBM copy; no
  collectives).
- Sparse i_rec = W @ z on-device per NC via GpSimd ap_gather: z is split into
  16 subtables of 15625 (one per partition within each 16-partition Q7 core
  group); edges are bucketed per (Q7-core k, lane j = col//15625) with per-row
  slot ranges padded to C(r) = max_j c_{r,j} so the segment structure is
  identical across the 16 lanes of a core (and, via a global class profile,
  across all cores and NCs — one SPMD program).  Per slot chunk: 16 ap_gather
  calls deliver z, 16 strided-partition SBUF DMAs assemble the useful
  diagonal, a weight-stream multiply and per-class strided tensor_reduce give
  per-lane row partials; a block-diagonal ones matmul on the PE sums the 16
  lanes of each core.
- All remaining GLIF dynamics are elementwise on [128, x] tiles.
- Host (numpy) work is limited to static graph restructuring (depends only on
  rec_idx), input layout permutation, and output reassembly.
"""
import sys, os, time
sys.path.insert(0, "/opt/trn_rl_repo")
import numpy as np
from contextlib import ExitStack

import jax
from jax.sharding import Mesh, PartitionSpec
from jax.experimental.shard_map import shard_map

import concourse.bass as bass
import concourse.tile as tile
from concourse import bacc, mybir
from concourse.bass2jax import _bass_exec_p, partition_id_tensor, install_neuronx_cc_hook

# ---------------- constants ----------------
N = 50000; R = 10; D = 5; E = 10_000_000
NC_N = 8            # NeuronCores
P = 128
NPC = N // NC_N     # neurons per NC = 6250
RPC = R * NPC       # rows per NC = 62500
ZT = D * N          # z entries = 250000
TSUB = ZT // 16     # subtable = 15625
F2 = 586            # phase-2 synapse tile free size
NRQ = 16 * F2       # padded layout rows per Q7 core = 9376 (128B-mult pitch)
FN = 49             # neuron tile free size (128*49 = 6272 >= 6250)
NNP = P * FN        # padded neurons = 6272
CH = 768            # gather slot chunk
DT = 1.0

_prep_cache = {}


def _fingerprint(rec_idx):
    a = np.ascontiguousarray(rec_idx)
    return (a.shape, a.dtype.str, hash(a[::997, 0].tobytes()),
            hash(a[:997, 1].tobytes()))


def _rank_within_group(key):
    """rank of each element within its equal-key group (stable)."""
    n = len(key)
    sidx = np.argsort(key, kind="stable")
    sk = key[sidx]
    first = np.r_[True, sk[1:] != sk[:-1]]
    grp_start_idx = np.nonzero(first)[0]
    grp_id = np.cumsum(first) - 1
    rank_sorted = np.arange(n) - grp_start_idx[grp_id]
    rank = np.empty(n, np.int64)
    rank[sidx] = rank_sorted
    return rank




def _balance_lanes(r_loc, col_g, rng_seed=12345):
    """Assign each z-column to one of 16 lanes minimizing sum_r max_j c_{r,j}.
    Returns lane[E], off[E], zperm [16, TSUB] (z index per (lane,off), -1 pad)."""
    rng = np.random.default_rng(rng_seed)
    deg = np.bincount(col_g, minlength=ZT)
    order = np.argsort(-deg, kind="stable")
    es = np.argsort(col_g, kind="stable")
    row_sorted = r_loc[es]
    starts = np.searchsorted(col_g[es], np.arange(ZT + 1))
    counts = np.zeros((RPC, 16), np.int32)
    loads = np.zeros(16, np.int64)
    lane_of_col = np.full(ZT, -1, np.int32)
    B = 8192
    for b0 in range(0, ZT, B):
        batch = order[b0:b0 + B]
        batch = batch[deg[batch] > 0]
        if len(batch) == 0:
            continue
        lens = deg[batch]
        tot = lens.sum()
        eidx = np.repeat(starts[batch], lens) + (
            np.arange(tot) - np.repeat(np.cumsum(lens) - lens, lens))
        rr = row_sorted[eidx]
        cloc = np.repeat(np.arange(len(batch)), lens)
        score = np.zeros((len(batch), 16), np.float64)
        np.add.at(score, cloc, counts[rr].astype(np.float64))
        score += rng.random(score.shape) * 0.01
        score[:, loads >= TSUB] = np.inf
        lane_b = np.argmin(score, 1).astype(np.int32)
        lane_of_col[batch] = lane_b
        np.add.at(counts, (rr, lane_b[cloc]), 1)
        loads += np.bincount(lane_b, minlength=16)
    z0 = np.nonzero(lane_of_col < 0)[0]
    free = np.repeat(np.arange(16), np.maximum(TSUB - loads, 0))
    lane_of_col[z0] = free[:len(z0)]
    # fix any lane overflow: move lowest-degree cols to under-full lanes
    loads = np.bincount(lane_of_col, minlength=16)
    if loads.max() > TSUB:
        for j in np.nonzero(loads > TSUB)[0]:
            cj = np.nonzero(lane_of_col == j)[0]
            cj = cj[np.argsort(deg[cj])]  # move low-degree first
            surplus = cj[:loads[j] - TSUB]
            for c in surplus:
                tgt = int(np.argmin(loads))
                lane_of_col[c] = tgt
                loads[tgt] += 1; loads[j] -= 1
    off_of_col = np.zeros(ZT, np.int64)
    zperm = np.full((16, TSUB), -1, np.int64)
    for j in range(16):
        cj = np.nonzero(lane_of_col == j)[0]
        off_of_col[cj] = np.arange(len(cj))
        zperm[j, :len(cj)] = cj
    return (lane_of_col[col_g].astype(np.int64), off_of_col[col_g], zperm)

def _prep_static(rec_idx, rec_w):
    rows_g = rec_idx[:, 0].astype(np.int64)
    cols_g = rec_idx[:, 1].astype(np.int64)
    w_all = np.asarray(rec_w, np.float32)
    nc_id = rows_g // RPC

    # pass 1: per-NC per-edge fields + global class profile
    percore = []
    for m in range(NC_N):
        em = np.nonzero(nc_id == m)[0]
        r_loc = rows_g[em] - m * RPC
        lane, off, zperm = _balance_lanes(r_loc, cols_g[em])
        w = w_all[em]
        counts = np.bincount(r_loc * 16 + lane, minlength=RPC * 16).reshape(RPC, 16)
        Cr = np.maximum(counts.max(1).astype(np.int64), 1)
        order = np.argsort(-Cr, kind="stable")
        snake = np.tile(np.r_[np.arange(8), np.arange(7, -1, -1)],
                        RPC // 16 + 1)[:RPC]
        core_of_row = np.empty(RPC, np.int64)
        core_of_row[order] = snake
        percore.append((r_loc, lane, off, w, Cr, core_of_row, zperm))

    cmax = int(max(p[4].max() for p in percore))
    # global per-class row-count profile: max over all (NC, core) pairs
    prof = np.zeros(cmax + 1, np.int64)
    for (r_loc, lane, off, w, Cr, corow, zperm) in percore:
        for k in range(8):
            sel = corow == k
            h = np.bincount(Cr[sel], minlength=cmax + 1)
            prof = np.maximum(prof, h)
    classes = [c for c in range(cmax, 0, -1) if prof[c] > 0]

    # shared layout: position/slot for every (class, i) with CH-straddle padding
    plan = []  # (class, rowpos0, nrows, slot0) each within one chunk
    pos = 0; slot = 0
    cls_bounds = {}  # class -> (pos0, nrows)
    for c in classes:
        nr_ = int(prof[c]); done = 0
        cls_bounds[c] = (pos, nr_)
        while done < nr_:
            room = (-slot) % CH
            if room == 0:
                room = CH
            fit = min(nr_ - done, room // c)
            if fit == 0:
                slot += room
                continue
            plan.append((int(c), int(pos), int(fit), int(slot)))
            pos += fit; slot += fit * c; done += fit
    NROW_USED = pos
    NSLOT = int(-(-slot // CH) * CH)
    NCHUNK = NSLOT // CH
    assert NROW_USED <= NRQ, (NROW_USED, NRQ)
    # slot base per layout position
    slot_of_pos = np.zeros(NROW_USED, np.int64)
    for (c, p0, nr_, s0) in plan:
        slot_of_pos[p0:p0 + nr_] = s0 + np.arange(nr_) * c

    out = []
    for m in range(NC_N):
        (r_loc, lane, off, w, Cr, corow, zperm) = percore[m]
        # per-core: assign rows of class c to positions cls_bounds[c]
        rowpos = np.full((8, RPC), -1, np.int64)
        layout_rows = np.full((8, NRQ), -1, np.int64)
        for k in range(8):
            rk = np.nonzero(corow == k)[0]
            crk = Cr[rk]
            ordk = np.argsort(-crk, kind="stable")
            rk = rk[ordk]; crk = crk[ordk]
            # positions: within class c, i-th row -> cls_bounds[c][0] + i
            ranks = _rank_within_group(-crk)
            posk = np.array([cls_bounds[int(c)][0] for c in crk]) + ranks
            rowpos[k, rk] = posk
            layout_rows[k, posk] = rk

        ek = corow[r_loc]
        epos = rowpos[ek, r_loc]
        rank = _rank_within_group(r_loc * 16 + lane)
        slot_e = slot_of_pos[epos] + rank

        O = np.zeros((8, 16, NSLOT), np.int16)
        W = np.zeros((8, 16, NSLOT), np.float32)
        O[ek, lane, slot_e] = off.astype(np.int16)
        W[ek, lane, slot_e] = w

        idxs = np.zeros((NCHUNK, P, 16, CH // 16), np.int16)
        Wd = np.zeros((NCHUNK, P, CH), np.float32)
        Ov = O.reshape(8, 16, NCHUNK, CH)
        Wv = W.reshape(8, 16, NCHUNK, CH)
        for k in range(8):
            for j in range(16):
                idxs[:, 16 * k:16 * k + 16, j, :] = (
                    Ov[k, j].reshape(NCHUNK, CH // 16, 16).transpose(0, 2, 1))
                Wd[:, 16 * k + j, :] = Wv[k, j]

        rho = np.full(8 * NRQ, -1, np.int64)
        for k in range(8):
            rho[k * NRQ:(k + 1) * NRQ] = layout_rows[k]
        out.append(dict(idxs=idxs, Wd=Wd, rho=rho, zperm=zperm))
    return dict(percore=out, plan=plan, NCHUNK=NCHUNK, NSLOT=NSLOT)


# ---------------- device kernel ----------------
def _build_kernel(NCHUNK, plan):
    nc = bacc.Bacc("TRN2", target_bir_lowering=False, debug=False,
                   num_devices=NC_N)
    f32, i16 = mybir.dt.float32, mybir.dt.int16
    DI = lambda n, s, d=f32: nc.dram_tensor(n, s, d, kind="ExternalInput").ap()
    DO = lambda n, s, d=f32: nc.dram_tensor(n, s, d, kind="ExternalOutput").ap()

    z2d_d = DI("z2d", [16, TSUB])
    idxs_d = DI("idxs", [NCHUNK, P, 16, CH // 16], i16)
    wstr_d = DI("wstr", [NCHUNK, P, CH])
    ones_d = DI("onesb", [P, 8])
    syn_d = DI("syn", [5, P, F2])     # inputs, psc_rise, psc, syn_decay, psc_initial
    pscn_d = DI("pscn", [P, FN * R])  # psc in neuron order (padded)
    nrn_d = DI("nrn", [18, P, FN])
    nz_d = DO("nz", [P, FN]); outv_d = DO("outv", [P, FN])
    nv_d = DO("nv", [P, FN]); nr_d = DO("nr", [P, FN])
    na1_d = DO("na1", [P, FN]); na2_d = DO("na2", [P, FN])
    nprise_d = DO("nprise", [P, F2]); npsc_d = DO("npsc", [P, F2])

    AluOp = mybir.AluOpType
    ActF = mybir.ActivationFunctionType

    with tile.TileContext(nc) as tc, ExitStack() as octx:
        opool = octx.enter_context(tc.tile_pool(name="outer", bufs=1))
        partial = opool.tile([P, NRQ], f32)
        nc.vector.memset(partial[:], 0.0)

        with ExitStack() as gctx:
            gpool = gctx.enter_context(tc.tile_pool(name="g", bufs=1))
            dbl = gctx.enter_context(tc.tile_pool(name="gdbl", bufs=2))
            ztab = gpool.tile([P, TSUB], f32)
            for k in range(8):
                nc.sync.dma_start(ztab[16 * k:16 * (k + 1), :], z2d_d[:])
            T16 = gpool.tile([P, 16, CH], f32)
            for c in range(NCHUNK):
                idxt = dbl.tile([P, 16 * (CH // 16)], i16, tag="idx")
                nc.sync.dma_start(idxt[:],
                                  idxs_d[c].rearrange("p j s -> p (j s)"))
                wt = dbl.tile([P, CH], f32, tag="w")
                nc.sync.dma_start(wt[:], wstr_d[c])
                nc.gpsimd.ap_gather(
                    out_ap=T16[:].rearrange("p j c -> p (j c)"), in_ap=ztab[:],
                    idxs_ap=idxt[:], channels=P, num_elems=TSUB, d=1,
                    num_idxs=16 * CH)
                V = dbl.tile([P, CH], f32, tag="v")
                for j in range(16):
                    nc.sync.dma_start(V[j:P:16, :], T16[j:P:16, j, :])
                nc.vector.tensor_mul(V[:], V[:], wt[:])
                for (cc, rowpos0, nrows, slot0) in plan:
                    if slot0 // CH != c:
                        continue
                    s0 = slot0 - c * CH
                    if cc == 1:
                        nc.vector.tensor_copy(
                            partial[:, rowpos0:rowpos0 + nrows],
                            V[:, s0:s0 + nrows])
                    else:
                        nc.vector.tensor_reduce(
                            partial[:, rowpos0:rowpos0 + nrows],
                            V[:, s0:s0 + nrows * cc].rearrange(
                                "p (n c) -> p n c", c=cc),
                            axis=mybir.AxisListType.X, op=AluOp.add)

        with ExitStack() as mctx:
            mpool = mctx.enter_context(tc.tile_pool(name="m", bufs=1))
            psum = mctx.enter_context(
                tc.tile_pool(name="ps", bufs=2, space="PSUM"))
            onesb = mpool.tile([P, 8], f32)
            nc.sync.dma_start(onesb[:], ones_d[:])
            irec8 = mpool.tile([8, NRQ], f32)
            t0 = 0
            while t0 < NRQ:
                csz = min(512, NRQ - t0)
                pt = psum.tile([8, 512], f32, space="PSUM", tag="pt")
                nc.tensor.matmul(out=pt[:, :csz], lhsT=onesb[:],
                                 rhs=partial[:, t0:t0 + csz],
                                 start=True, stop=True)
                nc.vector.tensor_copy(irec8[:, t0:t0 + csz], pt[:, :csz])
                t0 += csz
            # reshape [8, NRQ] -> [128, F2]: partition 16k+t <- irec8[k, t*F2:..]
            from concourse.ap import AP as RawAP
            i8ap = irec8[:]
            pitch = i8ap.ap[0][0]
            irec2 = mpool.tile([P, F2], f32)
            src = RawAP(tensor=i8ap.tensor, offset=i8ap.offset,
                        ap=[[pitch, 8], [F2, 16], [1, F2]])
            nc.sync.dma_start(irec2[:], src)

            # ---------------- phase 2 ----------------
            syn = [mpool.tile([P, F2], f32, tag=f"syn{i}", name=f"syn{i}") for i in range(5)]
            for i in range(5):
                nc.sync.dma_start(syn[i][:], syn_d[i])
            inp, prise, psc, sdec, pinit = syn
            pscn = mpool.tile([P, FN * R], f32)
            nc.sync.dma_start(pscn[:], pscn_d[:])
            nrn = [mpool.tile([P, FN], f32, tag=f"nrn{i}", name=f"nrn{i}") for i in range(18)]
            for i in range(18):
                nc.sync.dma_start(nrn[i][:], nrn_d[i])
            (v, r_, asc1, asc2, vth, el, vrst, g, dec, cf, tref,
             k0, k1, amp0, amp1, vscl, voff, pz) = nrn

            t1 = mpool.tile([P, F2], f32); t2 = mpool.tile([P, F2], f32)
            nc.vector.tensor_add(t1[:], irec2[:], inp[:])
            nc.vector.tensor_mul(t1[:], t1[:], pinit[:])
            nc.vector.tensor_mul(t2[:], sdec[:], prise[:])
            nc.vector.tensor_add(t1[:], t1[:], t2[:])
            nc.sync.dma_start(nprise_d[:], t1[:])
            nc.vector.tensor_add(t2[:], psc[:], prise[:])
            nc.vector.tensor_mul(t2[:], t2[:], sdec[:])
            nc.sync.dma_start(npsc_d[:], t2[:])

            a = mpool.tile([P, FN], f32); b = mpool.tile([P, FN], f32)
            c_ = mpool.tile([P, FN], f32); d_ = mpool.tile([P, FN], f32)
            cur = mpool.tile([P, FN], f32)
            nc.vector.tensor_reduce(
                cur[:], pscn[:].rearrange("p (n r) -> p n r", r=R),
                axis=mybir.AxisListType.X, op=AluOp.add)
            for (kk, ascx, amp, outd) in ((k0, asc1, amp0, na1_d),
                                          (k1, asc2, amp1, na2_d)):
                nc.scalar.activation(a[:], kk[:], ActF.Sigmoid)
                nc.scalar.activation(a[:], a[:], ActF.Exp, scale=-1.0)
                nc.vector.tensor_mul(a[:], a[:], ascx[:])
                nc.vector.tensor_mul(b[:], pz[:], amp[:])
                nc.vector.tensor_add(a[:], a[:], b[:])
                nc.sync.dma_start(outd[:], a[:])
            nc.vector.tensor_mul(a[:], pz[:], tref[:])
            nc.vector.tensor_add(a[:], a[:], r_[:])
            nc.vector.tensor_scalar_add(a[:], a[:], -DT)
            nc.vector.tensor_scalar_max(a[:], a[:], 0.0)
            nc.sync.dma_start(nr_d[:], a[:])          # a = new_r
            nc.vector.tensor_mul(b[:], g[:], el[:])
            nc.vector.tensor_add(b[:], b[:], cur[:])
            nc.vector.tensor_add(b[:], b[:], asc1[:])
            nc.vector.tensor_add(b[:], b[:], asc2[:])
            nc.vector.tensor_mul(b[:], b[:], cf[:])
            nc.vector.tensor_mul(c_[:], dec[:], v[:])
            nc.vector.tensor_add(b[:], b[:], c_[:])   # b = new_v (pre-reset)
            nc.vector.tensor_sub(c_[:], vth[:], el[:])
            nc.vector.reciprocal(c_[:], c_[:])
            nc.vector.tensor_sub(d_[:], b[:], vth[:])
            nc.vector.tensor_mul(d_[:], d_[:], c_[:])
            nc.vector.tensor_scalar(d_[:], d_[:], 0.0, None, op0=AluOp.is_gt)
            nc.vector.tensor_scalar(c_[:], a[:], 0.0, None, op0=AluOp.is_le)
            nc.vector.tensor_mul(d_[:], d_[:], c_[:])  # d_ = new_z
            nc.sync.dma_start(nz_d[:], d_[:])
            # blend: new_v = vrst*nz + b*(1-nz)
            nc.vector.tensor_scalar(t1[:, :FN], d_[:], -1.0, 1.0,
                                    op0=AluOp.mult, op1=AluOp.add)
            nc.vector.tensor_mul(b[:], b[:], t1[:, :FN])
            nc.vector.tensor_mul(c_[:], vrst[:], d_[:])
            nc.vector.tensor_add(c_[:], c_[:], b[:])
            nc.sync.dma_start(nv_d[:], c_[:])
            nc.vector.tensor_mul(c_[:], c_[:], vscl[:])
            nc.vector.tensor_add(c_[:], c_[:], voff[:])
            nc.sync.dma_start(outv_d[:], c_[:])
    nc.compile()
    return nc


# ---------------- runner ----------------
class _Runner:
    def __init__(self, nc, n_cores=NC_N):
        install_neuronx_cc_hook()
        self.nc = nc; self.n_cores = n_cores
        in_names, out_names, out_avals = [], [], []
        pname = nc.partition_id_tensor.name if nc.partition_id_tensor else None
        for alloc in nc.m.functions[0].allocations:
            if not isinstance(alloc, mybir.MemoryLocationSet):
                continue
            name = alloc.memorylocations[0].name
            if alloc.kind == "ExternalInput":
                if name != pname:
                    in_names.append(name)
            elif alloc.kind == "ExternalOutput":
                out_names.append(name)
                out_avals.append(jax.core.ShapedArray(
                    tuple(alloc.tensor_shape), mybir.dt.np(alloc.dtype)))
        self.in_names, self.out_names, self.out_avals = in_names, out_names, out_avals
        n_params = len(in_names); n_outs = len(out_avals)
        all_in = in_names + out_names + ([pname] if pname else [])
        donate = tuple(range(n_params, n_params + n_outs))
        use_pid = pname is not None

        def _body(*args):
            operands = list(args)
            if use_pid:
                operands.append(partition_id_tensor())
            return tuple(_bass_exec_p.bind(
                *operands, out_avals=tuple(out_avals), in_names=tuple(all_in),
                out_names=tuple(out_names), lowering_input_output_aliases=(),
                sim_require_finite=False, sim_require_nnan=False, nc=nc))

        devices = jax.devices()[:n_cores]
        mesh = Mesh(np.asarray(devices), ("core",))
        self.fn = jax.jit(
            shard_map(_body, mesh=mesh,
                      in_specs=(PartitionSpec("core"),) * (n_params + n_outs),
                      out_specs=(PartitionSpec("core"),) * n_outs,
                      check_rep=False),
            donate_argnums=donate, keep_unused=True)
        self.n_params = n_params

    def run(self, in_maps):
        per_core = [[np.asarray(m[n]) for n in self.in_names] for m in in_maps]
        cat = [np.concatenate([per_core[c][i] for c in range(self.n_cores)], axis=0)
               for i in range(self.n_params)]
        zeros = [np.zeros((self.n_cores * a.shape[0], *a.shape[1:]), a.dtype)
                 for a in self.out_avals]
        outs = self.fn(*cat, *zeros)
        jax.block_until_ready(outs)
        return [
            {n: np.asarray(outs[i]).reshape(self.n_cores,
                                            *self.out_avals[i].shape)[c]
             for i, n in enumerate(self.out_names)}
            for c in range(self.n_cores)
        ]


_kernel_cache = {}


def kernel(inputs, z_buf, v, r, asc_1, asc_2, psc_rise, psc, rec_w, rec_idx,
           v_th, e_l, v_reset, g, decay, current_factor, t_ref, k, asc_amps,
           syn_decay, psc_initial, voltage_scale, voltage_offset):
    fp = _fingerprint(np.asarray(rec_idx))
    if fp not in _prep_cache:
        _prep_cache[fp] = _prep_static(np.asarray(rec_idx), np.asarray(rec_w))
    prep = _prep_cache[fp]
    NCHUNK, plan = prep["NCHUNK"], prep["plan"]

    kkey = (NCHUNK, tuple(plan))
    if kkey not in _kernel_cache:
        _kernel_cache[kkey] = _Runner(_build_kernel(NCHUNK, plan))
    runner = _kernel_cache[kkey]

    onesb = np.zeros((P, 8), np.float32)
    for pp in range(P):
        onesb[pp, pp // 16] = 1.0
    z_full = np.asarray(z_buf, np.float32).reshape(ZT)

    in_maps = []
    for m in range(NC_N):
        pr = prep["percore"][m]
        zperm = pr["zperm"]
        z2d = np.where(zperm >= 0, z_full[np.maximum(zperm, 0)], 0.0
                       ).astype(np.float32)
        rho = pr["rho"]
        valid = rho >= 0
        gsl = slice(m * RPC, (m + 1) * RPC)
        nsl = slice(m * NPC, (m + 1) * NPC)

        def synp(x):
            x = np.asarray(x, np.float32).reshape(-1)[gsl]
            out = np.zeros(8 * NRQ, np.float32)
            out[valid] = x[rho[valid]]
            return out.reshape(P, F2)

        syn = np.stack([synp(inputs), synp(psc_rise), synp(psc),
                        synp(syn_decay), synp(psc_initial)])
        pscn = np.zeros(NNP * R, np.float32)
        pscn[:RPC] = np.asarray(psc, np.float32).reshape(-1)[gsl]
        nfield = lambda x: np.pad(np.asarray(x, np.float32).reshape(-1)[nsl],
                                  (0, NNP - NPC)).reshape(P, FN)
        k_ = np.asarray(k, np.float32); aa = np.asarray(asc_amps, np.float32)
        nrn = np.stack([
            nfield(v), nfield(r), nfield(asc_1), nfield(asc_2), nfield(v_th),
            nfield(e_l), nfield(v_reset), nfield(g), nfield(decay),
            nfield(current_factor), nfield(t_ref), nfield(k_[:, 0]),
            nfield(k_[:, 1]), nfield(aa[:, 0]), nfield(aa[:, 1]),
            nfield(voltage_scale), nfield(voltage_offset),
            nfield(z_full[:N]),
        ])
        in_maps.append(dict(z2d=z2d, idxs=pr["idxs"], wstr=pr["Wd"],
                            onesb=onesb, syn=syn,
                            pscn=pscn.reshape(P, FN * R), nrn=nrn))

    global _last
    _last = (runner, in_maps)
    res = runner.run(in_maps)

    new_z = np.zeros(N, np.float32); out_v = np.zeros(N, np.float32)
    new_v = np.zeros(N, np.float32); new_r = np.zeros(N, np.float32)
    na1 = np.zeros(N, np.float32); na2 = np.zeros(N, np.float32)
    nprise = np.zeros(R * N, np.float32); npsc = np.zeros(R * N, np.float32)
    for m in range(NC_N):
        o = res[m]
        nsl = slice(m * NPC, (m + 1) * NPC)
        gsl = slice(m * RPC, (m + 1) * RPC)
        new_z[nsl] = o["nz"].reshape(-1)[:NPC]
        out_v[nsl] = o["outv"].reshape(-1)[:NPC]
        new_v[nsl] = o["nv"].reshape(-1)[:NPC]
        new_r[nsl] = o["nr"].reshape(-1)[:NPC]
        na1[nsl] = o["na1"].reshape(-1)[:NPC]
        na2[nsl] = o["na2"].reshape(-1)[:NPC]
        rho = prep["percore"][m]["rho"]; valid = rho >= 0
        t = np.zeros(RPC, np.float32)
        t[rho[valid]] = o["nprise"].reshape(-1)[valid]
        nprise[gsl] = t
        t = np.zeros(RPC, np.float32)
        t[rho[valid]] = o["npsc"].reshape(-1)[valid]
        npsc[gsl] = t

    zb = np.asarray(z_buf, np.float32).reshape(1, ZT)
    new_z_buf = np.concatenate([new_z.reshape(1, N), zb[:, :(D - 1) * N]],
                               axis=1)
    return (new_z.reshape(1, N), out_v.reshape(1, N), new_z_buf,
            new_v.reshape(1, N), new_r.reshape(1, N), na1.reshape(1, N),
            na2.reshape(1, N), nprise.reshape(1, R * N), npsc.reshape(1, R * N))


_last = None


def hw_time_s(iters=5):
    """Device-execution wall estimate: jitted fn with device-resident inputs."""
    runner, in_maps = _last
    per_core = [[np.asarray(m[n]) for n in runner.in_names] for m in in_maps]
    cat = [np.concatenate([per_core[c][i] for c in range(runner.n_cores)], axis=0)
           for i in range(runner.n_params)]
    ins = [jax.device_put(x) for x in cat]
    jax.block_until_ready(ins)
    zeros = lambda: [np.zeros((runner.n_cores * a.shape[0], *a.shape[1:]), a.dtype)
                     for a in runner.out_avals]
    o = runner.fn(*ins, *zeros()); jax.block_until_ready(o)
    import time as _t
    ts = []
    for _ in range(iters):
        t0 = _t.perf_counter()
        o = runner.fn(*ins, *zeros()); jax.block_until_ready(o)
        ts.append(_t.perf_counter() - t0)
    return min(ts)


# revision 13
# speedup vs baseline: 1.0557x; 1.0454x over previous
"""BillehColumn single-step kernel on 8 Trainium2 NeuronCores.

Strategy (row/neuron partition, per sharding hint):
- NC m owns neurons [m*6250, (m+1)*6250) == synapse rows [m*62500, (m+1)*62500).
- Each NC reads the full delayed spike buffer z_buf (own HBM copy; no
  collectives).
- Sparse i_rec = W @ z on-device per NC via GpSimd ap_gather: z is split into
  16 subtables of 15625 (one per partition within each 16-partition Q7 core
  group); edges are bucketed per (Q7-core k, lane j = col//15625) with per-row
  slot ranges padded to C(r) = max_j c_{r,j} so the segment structure is
  identical across the 16 lanes of a core (and, via a global class profile,
  across all cores and NCs — one SPMD program).  Per slot chunk: 16 ap_gather
  calls deliver z, 16 strided-partition SBUF DMAs assemble the useful
  diagonal, a weight-stream multiply and per-class strided tensor_reduce give
  per-lane row partials; a block-diagonal ones matmul on the PE sums the 16
  lanes of each core.
- All remaining GLIF dynamics are elementwise on [128, x] tiles.
- Host (numpy) work is limited to static graph restructuring (depends only on
  rec_idx), input layout permutation, and output reassembly.
"""
import sys, os, time
sys.path.insert(0, "/opt/trn_rl_repo")
import numpy as np
from contextlib import ExitStack

import jax
from jax.sharding import Mesh, PartitionSpec
from jax.experimental.shard_map import shard_map

import concourse.bass as bass
import concourse.tile as tile
from concourse import bacc, mybir
from concourse.bass2jax import _bass_exec_p, partition_id_tensor, install_neuronx_cc_hook

# ---------------- constants ----------------
N = 50000; R = 10; D = 5; E = 10_000_000
NC_N = 8            # NeuronCores
P = 128
NPC = N // NC_N     # neurons per NC = 6250
RPC = R * NPC       # rows per NC = 62500
ZT = D * N          # z entries = 250000
TSUB = ZT // 16     # subtable = 15625
F2 = 586            # phase-2 synapse tile free size
NRQ = 16 * F2       # padded layout rows per Q7 core = 9376 (128B-mult pitch)
FN = 49             # neuron tile free size (128*49 = 6272 >= 6250)
NNP = P * FN        # padded neurons = 6272
CH = 768            # gather slot chunk
DT = 1.0

_prep_cache = {}


def _fingerprint(rec_idx):
    a = np.ascontiguousarray(rec_idx)
    return (a.shape, a.dtype.str, hash(a[::997, 0].tobytes()),
            hash(a[:997, 1].tobytes()))


def _rank_within_group(key):
    """rank of each element within its equal-key group (stable)."""
    n = len(key)
    sidx = np.argsort(key, kind="stable")
    sk = key[sidx]
    first = np.r_[True, sk[1:] != sk[:-1]]
    grp_start_idx = np.nonzero(first)[0]
    grp_id = np.cumsum(first) - 1
    rank_sorted = np.arange(n) - grp_start_idx[grp_id]
    rank = np.empty(n, np.int64)
    rank[sidx] = rank_sorted
    return rank




def _balance_lanes(r_loc, col_g, rng_seed=12345):
    """Assign each z-column to one of 16 lanes minimizing sum_r max_j c_{r,j}.
    Returns lane[E], off[E], zperm [16, TSUB] (z index per (lane,off), -1 pad)."""
    rng = np.random.default_rng(rng_seed)
    deg = np.bincount(col_g, minlength=ZT)
    order = np.argsort(-deg, kind="stable")
    es = np.argsort(col_g, kind="stable")
    row_sorted = r_loc[es]
    starts = np.searchsorted(col_g[es], np.arange(ZT + 1))
    counts = np.zeros((RPC, 16), np.int32)
    loads = np.zeros(16, np.int64)
    lane_of_col = np.full(ZT, -1, np.int32)
    B = 8192
    for b0 in range(0, ZT, B):
        batch = order[b0:b0 + B]
        batch = batch[deg[batch] > 0]
        if len(batch) == 0:
            continue
        lens = deg[batch]
        tot = lens.sum()
        eidx = np.repeat(starts[batch], lens) + (
            np.arange(tot) - np.repeat(np.cumsum(lens) - lens, lens))
        rr = row_sorted[eidx]
        cloc = np.repeat(np.arange(len(batch)), lens)
        score = np.zeros((len(batch), 16), np.float64)
        np.add.at(score, cloc, counts[rr].astype(np.float64))
        score += rng.random(score.shape) * 0.01
        score[:, loads >= TSUB] = np.inf
        lane_b = np.argmin(score, 1).astype(np.int32)
        lane_of_col[batch] = lane_b
        np.add.at(counts, (rr, lane_b[cloc]), 1)
        loads += np.bincount(lane_b, minlength=16)
    z0 = np.nonzero(lane_of_col < 0)[0]
    free = np.repeat(np.arange(16), np.maximum(TSUB - loads, 0))
    lane_of_col[z0] = free[:len(z0)]
    # fix any lane overflow: move lowest-degree cols to under-full lanes
    loads = np.bincount(lane_of_col, minlength=16)
    if loads.max() > TSUB:
        for j in np.nonzero(loads > TSUB)[0]:
            cj = np.nonzero(lane_of_col == j)[0]
            cj = cj[np.argsort(deg[cj])]  # move low-degree first
            surplus = cj[:loads[j] - TSUB]
            for c in surplus:
                tgt = int(np.argmin(loads))
                lane_of_col[c] = tgt
                loads[tgt] += 1; loads[j] -= 1
    off_of_col = np.zeros(ZT, np.int64)
    zperm = np.full((16, TSUB), -1, np.int64)
    for j in range(16):
        cj = np.nonzero(lane_of_col == j)[0]
        off_of_col[cj] = np.arange(len(cj))
        zperm[j, :len(cj)] = cj
    return (lane_of_col[col_g].astype(np.int64), off_of_col[col_g], zperm)

def _prep_static(rec_idx, rec_w):
    rows_g = rec_idx[:, 0].astype(np.int64)
    cols_g = rec_idx[:, 1].astype(np.int64)
    w_all = np.asarray(rec_w, np.float32)
    nc_id = rows_g // RPC

    # pass 1: per-NC per-edge fields + global class profile
    percore = []
    for m in range(NC_N):
        em = np.nonzero(nc_id == m)[0]
        r_loc = rows_g[em] - m * RPC
        lane, off, zperm = _balance_lanes(r_loc, cols_g[em])
        w = w_all[em]
        counts = np.bincount(r_loc * 16 + lane, minlength=RPC * 16).reshape(RPC, 16)
        Cr = np.maximum(counts.max(1).astype(np.int64), 1)
        order = np.argsort(-Cr, kind="stable")
        snake = np.tile(np.r_[np.arange(8), np.arange(7, -1, -1)],
                        RPC // 16 + 1)[:RPC]
        core_of_row = np.empty(RPC, np.int64)
        core_of_row[order] = snake
        percore.append((r_loc, lane, off, w, Cr, core_of_row, zperm))

    cmax = int(max(p[4].max() for p in percore))
    # global per-class row-count profile: max over all (NC, core) pairs
    prof = np.zeros(cmax + 1, np.int64)
    for (r_loc, lane, off, w, Cr, corow, zperm) in percore:
        for k in range(8):
            sel = corow == k
            h = np.bincount(Cr[sel], minlength=cmax + 1)
            prof = np.maximum(prof, h)
    classes = [c for c in range(cmax, 0, -1) if prof[c] > 0]

    # shared layout: position/slot for every (class, i) with CH-straddle padding
    plan = []  # (class, rowpos0, nrows, slot0) each within one chunk
    pos = 0; slot = 0
    cls_bounds = {}  # class -> (pos0, nrows)
    for c in classes:
        nr_ = int(prof[c]); done = 0
        cls_bounds[c] = (pos, nr_)
        while done < nr_:
            room = (-slot) % CH
            if room == 0:
                room = CH
            fit = min(nr_ - done, room // c)
            if fit == 0:
                slot += room
                continue
            plan.append((int(c), int(pos), int(fit), int(slot)))
            pos += fit; slot += fit * c; done += fit
    NROW_USED = pos
    NSLOT = int(-(-slot // CH) * CH)
    NCHUNK = NSLOT // CH
    assert NROW_USED <= NRQ, (NROW_USED, NRQ)
    # slot base per layout position
    slot_of_pos = np.zeros(NROW_USED, np.int64)
    for (c, p0, nr_, s0) in plan:
        slot_of_pos[p0:p0 + nr_] = s0 + np.arange(nr_) * c

    out = []
    for m in range(NC_N):
        (r_loc, lane, off, w, Cr, corow, zperm) = percore[m]
        # per-core: assign rows of class c to positions cls_bounds[c]
        rowpos = np.full((8, RPC), -1, np.int64)
        layout_rows = np.full((8, NRQ), -1, np.int64)
        for k in range(8):
            rk = np.nonzero(corow == k)[0]
            crk = Cr[rk]
            ordk = np.argsort(-crk, kind="stable")
            rk = rk[ordk]; crk = crk[ordk]
            # positions: within class c, i-th row -> cls_bounds[c][0] + i
            ranks = _rank_within_group(-crk)
            posk = np.array([cls_bounds[int(c)][0] for c in crk]) + ranks
            rowpos[k, rk] = posk
            layout_rows[k, posk] = rk

        ek = corow[r_loc]
        epos = rowpos[ek, r_loc]
        rank = _rank_within_group(r_loc * 16 + lane)
        slot_e = slot_of_pos[epos] + rank

        O = np.zeros((8, 16, NSLOT), np.int16)
        W = np.zeros((8, 16, NSLOT), np.float32)
        O[ek, lane, slot_e] = off.astype(np.int16)
        W[ek, lane, slot_e] = w

        idxs = np.zeros((NCHUNK, P, 16, CH // 16), np.int16)
        Wd = np.zeros((NCHUNK, P, CH), np.float32)
        Ov = O.reshape(8, 16, NCHUNK, CH)
        Wv = W.reshape(8, 16, NCHUNK, CH)
        for k in range(8):
            for j in range(16):
                idxs[:, 16 * k:16 * k + 16, j, :] = (
                    Ov[k, j].reshape(NCHUNK, CH // 16, 16).transpose(0, 2, 1))
                Wd[:, 16 * k + j, :] = Wv[k, j]

        rho = np.full(8 * NRQ, -1, np.int64)
        for k in range(8):
            rho[k * NRQ:(k + 1) * NRQ] = layout_rows[k]
        out.append(dict(idxs=idxs, Wd=Wd, rho=rho, zperm=zperm))
    return dict(percore=out, plan=plan, NCHUNK=NCHUNK, NSLOT=NSLOT)


# ---------------- device kernel ----------------
def _build_kernel(NCHUNK, plan):
    nc = bacc.Bacc("TRN2", target_bir_lowering=False, debug=False,
                   num_devices=NC_N)
    f32, i16 = mybir.dt.float32, mybir.dt.int16
    DI = lambda n, s, d=f32: nc.dram_tensor(n, s, d, kind="ExternalInput").ap()
    DO = lambda n, s, d=f32: nc.dram_tensor(n, s, d, kind="ExternalOutput").ap()

    z2d_d = DI("z2d", [16, TSUB])
    idxs_d = DI("idxs", [NCHUNK, P, 16, CH // 16], i16)
    wstr_d = DI("wstr", [NCHUNK, P, CH])
    ones_d = DI("onesb", [P, 8])
    syn_d = DI("syn", [5, P, F2])     # inputs, psc_rise, psc, syn_decay, psc_initial
    pscn_d = DI("pscn", [P, FN * R])  # psc in neuron order (padded)
    nrn_d = DI("nrn", [18, P, FN])
    nz_d = DO("nz", [P, FN]); outv_d = DO("outv", [P, FN])
    nv_d = DO("nv", [P, FN]); nr_d = DO("nr", [P, FN])
    na1_d = DO("na1", [P, FN]); na2_d = DO("na2", [P, FN])
    nprise_d = DO("nprise", [P, F2]); npsc_d = DO("npsc", [P, F2])

    AluOp = mybir.AluOpType
    ActF = mybir.ActivationFunctionType

    with tile.TileContext(nc) as tc, ExitStack() as octx:
        opool = octx.enter_context(tc.tile_pool(name="outer", bufs=1))
        partial = opool.tile([P, NRQ], f32)
        nc.vector.memset(partial[:], 0.0)

        with ExitStack() as gctx:
            gpool = gctx.enter_context(tc.tile_pool(name="g", bufs=1))
            dbl = gctx.enter_context(tc.tile_pool(name="gdbl", bufs=2))
            ztab = gpool.tile([P, TSUB], f32)
            for k in range(8):
                nc.sync.dma_start(ztab[16 * k:16 * (k + 1), :], z2d_d[:])
            T16 = gpool.tile([P, 16, CH], f32)
            for c in range(NCHUNK):
                idxt = dbl.tile([P, 16 * (CH // 16)], i16, tag="idx")
                nc.sync.dma_start(idxt[:],
                                  idxs_d[c].rearrange("p j s -> p (j s)"))
                wt = dbl.tile([P, CH], f32, tag="w")
                nc.sync.dma_start(wt[:], wstr_d[c])
                nc.gpsimd.ap_gather(
                    out_ap=T16[:].rearrange("p j c -> p (j c)"), in_ap=ztab[:],
                    idxs_ap=idxt[:], channels=P, num_elems=TSUB, d=1,
                    num_idxs=16 * CH)
                V = dbl.tile([P, CH], f32, tag="v")
                for j in range(16):
                    nc.sync.dma_start(V[j:P:16, :], T16[j:P:16, j, :])
                nc.vector.tensor_mul(V[:], V[:], wt[:])
                for (cc, rowpos0, nrows, slot0) in plan:
                    if slot0 // CH != c:
                        continue
                    s0 = slot0 - c * CH
                    if cc == 1:
                        nc.vector.tensor_copy(
                            partial[:, rowpos0:rowpos0 + nrows],
                            V[:, s0:s0 + nrows])
                    else:
                        nc.vector.tensor_reduce(
                            partial[:, rowpos0:rowpos0 + nrows],
                            V[:, s0:s0 + nrows * cc].rearrange(
                                "p (n c) -> p n c", c=cc),
                            axis=mybir.AxisListType.X, op=AluOp.add)

        with ExitStack() as mctx:
            mpool = mctx.enter_context(tc.tile_pool(name="m", bufs=1))
            psum = mctx.enter_context(
                tc.tile_pool(name="ps", bufs=2, space="PSUM"))
            onesb = mpool.tile([P, 8], f32)
            nc.sync.dma_start(onesb[:], ones_d[:])
            irec8 = mpool.tile([8, NRQ], f32)
            t0 = 0
            while t0 < NRQ:
                csz = min(512, NRQ - t0)
                pt = psum.tile([8, 512], f32, space="PSUM", tag="pt")
                nc.tensor.matmul(out=pt[:, :csz], lhsT=onesb[:],
                                 rhs=partial[:, t0:t0 + csz],
                                 start=True, stop=True)
                nc.vector.tensor_copy(irec8[:, t0:t0 + csz], pt[:, :csz])
                t0 += csz
            # reshape [8, NRQ] -> [128, F2]: partition 16k+t <- irec8[k, t*F2:..]
            from concourse.ap import AP as RawAP
            i8ap = irec8[:]
            pitch = i8ap.ap[0][0]
            irec2 = mpool.tile([P, F2], f32)
            src = RawAP(tensor=i8ap.tensor, offset=i8ap.offset,
                        ap=[[pitch, 8], [F2, 16], [1, F2]])
            nc.sync.dma_start(irec2[:], src)

            # ---------------- phase 2 ----------------
            syn = [mpool.tile([P, F2], f32, tag=f"syn{i}", name=f"syn{i}") for i in range(5)]
            for i in range(5):
                nc.sync.dma_start(syn[i][:], syn_d[i])
            inp, prise, psc, sdec, pinit = syn
            pscn = mpool.tile([P, FN * R], f32)
            nc.sync.dma_start(pscn[:], pscn_d[:])
            nrn = [mpool.tile([P, FN], f32, tag=f"nrn{i}", name=f"nrn{i}") for i in range(18)]
            for i in range(18):
                nc.sync.dma_start(nrn[i][:], nrn_d[i])
            (v, r_, asc1, asc2, vth, el, vrst, g, dec, cf, tref,
             k0, k1, amp0, amp1, vscl, voff, pz) = nrn

            t1 = mpool.tile([P, F2], f32); t2 = mpool.tile([P, F2], f32)
            nc.vector.tensor_add(t1[:], irec2[:], inp[:])
            nc.vector.tensor_mul(t1[:], t1[:], pinit[:])
            nc.vector.tensor_mul(t2[:], sdec[:], prise[:])
            nc.vector.tensor_add(t1[:], t1[:], t2[:])
            nc.sync.dma_start(nprise_d[:], t1[:])
            nc.vector.tensor_add(t2[:], psc[:], prise[:])
            nc.vector.tensor_mul(t2[:], t2[:], sdec[:])
            nc.sync.dma_start(npsc_d[:], t2[:])

            a = mpool.tile([P, FN], f32); b = mpool.tile([P, FN], f32)
            c_ = mpool.tile([P, FN], f32); d_ = mpool.tile([P, FN], f32)
            cur = mpool.tile([P, FN], f32)
            nc.vector.tensor_reduce(
                cur[:], pscn[:].rearrange("p (n r) -> p n r", r=R),
                axis=mybir.AxisListType.X, op=AluOp.add)
            for (kk, ascx, amp, outd) in ((k0, asc1, amp0, na1_d),
                                          (k1, asc2, amp1, na2_d)):
                nc.scalar.activation(a[:], kk[:], ActF.Sigmoid)
                nc.scalar.activation(a[:], a[:], ActF.Exp, scale=-1.0)
                nc.vector.tensor_mul(a[:], a[:], ascx[:])
                nc.vector.tensor_mul(b[:], pz[:], amp[:])
                nc.vector.tensor_add(a[:], a[:], b[:])
                nc.sync.dma_start(outd[:], a[:])
            nc.vector.tensor_mul(a[:], pz[:], tref[:])
            nc.vector.tensor_add(a[:], a[:], r_[:])
            nc.vector.tensor_scalar_add(a[:], a[:], -DT)
            nc.vector.tensor_scalar_max(a[:], a[:], 0.0)
            nc.sync.dma_start(nr_d[:], a[:])          # a = new_r
            nc.vector.tensor_mul(b[:], g[:], el[:])
            nc.vector.tensor_add(b[:], b[:], cur[:])
            nc.vector.tensor_add(b[:], b[:], asc1[:])
            nc.vector.tensor_add(b[:], b[:], asc2[:])
            nc.vector.tensor_mul(b[:], b[:], cf[:])
            nc.vector.tensor_mul(c_[:], dec[:], v[:])
            nc.vector.tensor_add(b[:], b[:], c_[:])   # b = new_v (pre-reset)
            nc.vector.tensor_sub(c_[:], vth[:], el[:])
            nc.vector.reciprocal(c_[:], c_[:])
            nc.vector.tensor_sub(d_[:], b[:], vth[:])
            nc.vector.tensor_mul(d_[:], d_[:], c_[:])
            nc.vector.tensor_scalar(d_[:], d_[:], 0.0, None, op0=AluOp.is_gt)
            nc.vector.tensor_scalar(c_[:], a[:], 0.0, None, op0=AluOp.is_le)
            nc.vector.tensor_mul(d_[:], d_[:], c_[:])  # d_ = new_z
            nc.sync.dma_start(nz_d[:], d_[:])
            # blend: new_v = vrst*nz + b*(1-nz)
            nc.vector.tensor_scalar(t1[:, :FN], d_[:], -1.0, 1.0,
                                    op0=AluOp.mult, op1=AluOp.add)
            nc.vector.tensor_mul(b[:], b[:], t1[:, :FN])
            nc.vector.tensor_mul(c_[:], vrst[:], d_[:])
            nc.vector.tensor_add(c_[:], c_[:], b[:])
            nc.sync.dma_start(nv_d[:], c_[:])
            nc.vector.tensor_mul(c_[:], c_[:], vscl[:])
            nc.vector.tensor_add(c_[:], c_[:], voff[:])
            nc.sync.dma_start(outv_d[:], c_[:])
    nc.compile()
    return nc


# ---------------- runner ----------------
class _Runner:
    def __init__(self, nc, n_cores=NC_N):
        install_neuronx_cc_hook()
        self.nc = nc; self.n_cores = n_cores
        in_names, out_names, out_avals = [], [], []
        pname = nc.partition_id_tensor.name if nc.partition_id_tensor else None
        for alloc in nc.m.functions[0].allocations:
            if not isinstance(alloc, mybir.MemoryLocationSet):
                continue
            name = alloc.memorylocations[0].name
            if alloc.kind == "ExternalInput":
                if name != pname:
                    in_names.append(name)
            elif alloc.kind == "ExternalOutput":
                out_names.append(name)
                out_avals.append(jax.core.ShapedArray(
                    tuple(alloc.tensor_shape), mybir.dt.np(alloc.dtype)))
        self.in_names, self.out_names, self.out_avals = in_names, out_names, out_avals
        n_params = len(in_names); n_outs = len(out_avals)
        all_in = in_names + out_names + ([pname] if pname else [])
        donate = tuple(range(n_params, n_params + n_outs))
        use_pid = pname is not None

        def _body(*args):
            operands = list(args)
            if use_pid:
                operands.append(partition_id_tensor())
            return tuple(_bass_exec_p.bind(
                *operands, out_avals=tuple(out_avals), in_names=tuple(all_in),
                out_names=tuple(out_names), lowering_input_output_aliases=(),
                sim_require_finite=False, sim_require_nnan=False, nc=nc))

        devices = jax.devices()[:n_cores]
        mesh = Mesh(np.asarray(devices), ("core",))
        self.mesh = mesh
        self.fn = jax.jit(
            shard_map(_body, mesh=mesh,
                      in_specs=(PartitionSpec("core"),) * (n_params + n_outs),
                      out_specs=(PartitionSpec("core"),) * n_outs,
                      check_rep=False),
            donate_argnums=donate, keep_unused=True)
        self.n_params = n_params

    def run(self, in_maps):
        per_core = [[np.asarray(m[n]) for n in self.in_names] for m in in_maps]
        cat = [np.concatenate([per_core[c][i] for c in range(self.n_cores)], axis=0)
               for i in range(self.n_params)]
        zeros = [np.zeros((self.n_cores * a.shape[0], *a.shape[1:]), a.dtype)
                 for a in self.out_avals]
        outs = self.fn(*cat, *zeros)
        jax.block_until_ready(outs)
        return [
            {n: np.asarray(outs[i]).reshape(self.n_cores,
                                            *self.out_avals[i].shape)[c]
             for i, n in enumerate(self.out_names)}
            for c in range(self.n_cores)
        ]


_kernel_cache = {}


def kernel(inputs, z_buf, v, r, asc_1, asc_2, psc_rise, psc, rec_w, rec_idx,
           v_th, e_l, v_reset, g, decay, current_factor, t_ref, k, asc_amps,
           syn_decay, psc_initial, voltage_scale, voltage_offset):
    fp = _fingerprint(np.asarray(rec_idx))
    if fp not in _prep_cache:
        _prep_cache[fp] = _prep_static(np.asarray(rec_idx), np.asarray(rec_w))
    prep = _prep_cache[fp]
    NCHUNK, plan = prep["NCHUNK"], prep["plan"]

    kkey = (NCHUNK, tuple(plan))
    if kkey not in _kernel_cache:
        _kernel_cache[kkey] = _Runner(_build_kernel(NCHUNK, plan))
    runner = _kernel_cache[kkey]

    onesb = np.zeros((P, 8), np.float32)
    for pp in range(P):
        onesb[pp, pp // 16] = 1.0
    z_full = np.asarray(z_buf, np.float32).reshape(ZT)

    in_maps = []
    for m in range(NC_N):
        pr = prep["percore"][m]
        zperm = pr["zperm"]
        z2d = np.where(zperm >= 0, z_full[np.maximum(zperm, 0)], 0.0
                       ).astype(np.float32)
        rho = pr["rho"]
        valid = rho >= 0
        gsl = slice(m * RPC, (m + 1) * RPC)
        nsl = slice(m * NPC, (m + 1) * NPC)

        def synp(x):
            x = np.asarray(x, np.float32).reshape(-1)[gsl]
            out = np.zeros(8 * NRQ, np.float32)
            out[valid] = x[rho[valid]]
            return out.reshape(P, F2)

        syn = np.stack([synp(inputs), synp(psc_rise), synp(psc),
                        synp(syn_decay), synp(psc_initial)])
        pscn = np.zeros(NNP * R, np.float32)
        pscn[:RPC] = np.asarray(psc, np.float32).reshape(-1)[gsl]
        nfield = lambda x: np.pad(np.asarray(x, np.float32).reshape(-1)[nsl],
                                  (0, NNP - NPC)).reshape(P, FN)
        k_ = np.asarray(k, np.float32); aa = np.asarray(asc_amps, np.float32)
        nrn = np.stack([
            nfield(v), nfield(r), nfield(asc_1), nfield(asc_2), nfield(v_th),
            nfield(e_l), nfield(v_reset), nfield(g), nfield(decay),
            nfield(current_factor), nfield(t_ref), nfield(k_[:, 0]),
            nfield(k_[:, 1]), nfield(aa[:, 0]), nfield(aa[:, 1]),
            nfield(voltage_scale), nfield(voltage_offset),
            nfield(z_full[:N]),
        ])
        in_maps.append(dict(z2d=z2d, idxs=pr["idxs"], wstr=pr["Wd"],
                            onesb=onesb, syn=syn,
                            pscn=pscn.reshape(P, FN * R), nrn=nrn))

    global _last
    _last = (runner, in_maps)
    res = runner.run(in_maps)

    new_z = np.zeros(N, np.float32); out_v = np.zeros(N, np.float32)
    new_v = np.zeros(N, np.float32); new_r = np.zeros(N, np.float32)
    na1 = np.zeros(N, np.float32); na2 = np.zeros(N, np.float32)
    nprise = np.zeros(R * N, np.float32); npsc = np.zeros(R * N, np.float32)
    for m in range(NC_N):
        o = res[m]
        nsl = slice(m * NPC, (m + 1) * NPC)
        gsl = slice(m * RPC, (m + 1) * RPC)
        new_z[nsl] = o["nz"].reshape(-1)[:NPC]
        out_v[nsl] = o["outv"].reshape(-1)[:NPC]
        new_v[nsl] = o["nv"].reshape(-1)[:NPC]
        new_r[nsl] = o["nr"].reshape(-1)[:NPC]
        na1[nsl] = o["na1"].reshape(-1)[:NPC]
        na2[nsl] = o["na2"].reshape(-1)[:NPC]
        rho = prep["percore"][m]["rho"]; valid = rho >= 0
        t = np.zeros(RPC, np.float32)
        t[rho[valid]] = o["nprise"].reshape(-1)[valid]
        nprise[gsl] = t
        t = np.zeros(RPC, np.float32)
        t[rho[valid]] = o["npsc"].reshape(-1)[valid]
        npsc[gsl] = t

    zb = np.asarray(z_buf, np.float32).reshape(1, ZT)
    new_z_buf = np.concatenate([new_z.reshape(1, N), zb[:, :(D - 1) * N]],
                               axis=1)
    return (new_z.reshape(1, N), out_v.reshape(1, N), new_z_buf,
            new_v.reshape(1, N), new_r.reshape(1, N), na1.reshape(1, N),
            na2.reshape(1, N), nprise.reshape(1, R * N), npsc.reshape(1, R * N))


_last = None


def hw_time_s(iters=5):
    """Device-execution wall estimate: jitted fn with device-resident inputs."""
    runner, in_maps = _last
    per_core = [[np.asarray(m[n]) for n in runner.in_names] for m in in_maps]
    cat = [np.concatenate([per_core[c][i] for c in range(runner.n_cores)], axis=0)
           for i in range(runner.n_params)]
    from jax.sharding import NamedSharding
    sh = NamedSharding(runner.mesh, PartitionSpec("core"))
    ins = [jax.device_put(x, sh) for x in cat]
    jax.block_until_ready(ins)
    zeros = lambda: [np.zeros((runner.n_cores * a.shape[0], *a.shape[1:]), a.dtype)
                     for a in runner.out_avals]
    o = runner.fn(*ins, *zeros()); jax.block_until_ready(o)
    import time as _t
    ts = []
    for _ in range(iters):
        t0 = _t.perf_counter()
        o = runner.fn(*ins, *zeros()); jax.block_until_ready(o)
        ts.append(_t.perf_counter() - t0)
    return min(ts)


# revision 15
# speedup vs baseline: 1.1935x; 1.1305x over previous
"""BillehColumn single-step kernel on 8 Trainium2 NeuronCores.

Strategy (row/neuron partition, per sharding hint):
- NC m owns neurons [m*6250, (m+1)*6250) == synapse rows [m*62500, (m+1)*62500).
- Each NC reads the full delayed spike buffer z_buf (own HBM copy; no
  collectives).
- Sparse i_rec = W @ z on-device per NC via GpSimd ap_gather: z is split into
  16 subtables of 15625 (one per partition within each 16-partition Q7 core
  group); edges are bucketed per (Q7-core k, lane j = col//15625) with per-row
  slot ranges padded to C(r) = max_j c_{r,j} so the segment structure is
  identical across the 16 lanes of a core (and, via a global class profile,
  across all cores and NCs — one SPMD program).  Per slot chunk: 16 ap_gather
  calls deliver z, 16 strided-partition SBUF DMAs assemble the useful
  diagonal, a weight-stream multiply and per-class strided tensor_reduce give
  per-lane row partials; a block-diagonal ones matmul on the PE sums the 16
  lanes of each core.
- All remaining GLIF dynamics are elementwise on [128, x] tiles.
- Host (numpy) work is limited to static graph restructuring (depends only on
  rec_idx), input layout permutation, and output reassembly.
"""
import sys, os, time
sys.path.insert(0, "/opt/trn_rl_repo")
import numpy as np
from contextlib import ExitStack

import jax
from jax.sharding import Mesh, PartitionSpec
from jax.experimental.shard_map import shard_map

import concourse.bass as bass
import concourse.tile as tile
from concourse import bacc, mybir
from concourse.bass2jax import _bass_exec_p, partition_id_tensor, install_neuronx_cc_hook

# ---------------- constants ----------------
N = 50000; R = 10; D = 5; E = 10_000_000
NC_N = 8            # NeuronCores
P = 128
NPC = N // NC_N     # neurons per NC = 6250
RPC = R * NPC       # rows per NC = 62500
ZT = D * N          # z entries = 250000
TSUB = ZT // 16     # subtable = 15625
F2 = 586            # phase-2 synapse tile free size
NRQ = 16 * F2       # padded layout rows per Q7 core = 9376 (128B-mult pitch)
FN = 49             # neuron tile free size (128*49 = 6272 >= 6250)
NNP = P * FN        # padded neurons = 6272
CH = 512            # gather slot chunk
DT = 1.0

_prep_cache = {}


def _fingerprint(rec_idx):
    a = np.ascontiguousarray(rec_idx)
    return (a.shape, a.dtype.str, hash(a[::997, 0].tobytes()),
            hash(a[:997, 1].tobytes()))


def _rank_within_group(key):
    """rank of each element within its equal-key group (stable)."""
    n = len(key)
    sidx = np.argsort(key, kind="stable")
    sk = key[sidx]
    first = np.r_[True, sk[1:] != sk[:-1]]
    grp_start_idx = np.nonzero(first)[0]
    grp_id = np.cumsum(first) - 1
    rank_sorted = np.arange(n) - grp_start_idx[grp_id]
    rank = np.empty(n, np.int64)
    rank[sidx] = rank_sorted
    return rank




def _balance_lanes(r_loc, col_g, rng_seed=12345):
    """Assign each z-column to one of 16 lanes minimizing sum_r max_j c_{r,j}.
    Returns lane[E], off[E], zperm [16, TSUB] (z index per (lane,off), -1 pad)."""
    rng = np.random.default_rng(rng_seed)
    deg = np.bincount(col_g, minlength=ZT)
    order = np.argsort(-deg, kind="stable")
    es = np.argsort(col_g, kind="stable")
    row_sorted = r_loc[es]
    starts = np.searchsorted(col_g[es], np.arange(ZT + 1))
    counts = np.zeros((RPC, 16), np.int32)
    loads = np.zeros(16, np.int64)
    lane_of_col = np.full(ZT, -1, np.int32)
    B = 8192
    for b0 in range(0, ZT, B):
        batch = order[b0:b0 + B]
        batch = batch[deg[batch] > 0]
        if len(batch) == 0:
            continue
        lens = deg[batch]
        tot = lens.sum()
        eidx = np.repeat(starts[batch], lens) + (
            np.arange(tot) - np.repeat(np.cumsum(lens) - lens, lens))
        rr = row_sorted[eidx]
        cloc = np.repeat(np.arange(len(batch)), lens)
        score = np.zeros((len(batch), 16), np.float64)
        np.add.at(score, cloc, counts[rr].astype(np.float64))
        score += rng.random(score.shape) * 0.01
        score[:, loads >= TSUB] = np.inf
        lane_b = np.argmin(score, 1).astype(np.int32)
        lane_of_col[batch] = lane_b
        np.add.at(counts, (rr, lane_b[cloc]), 1)
        loads += np.bincount(lane_b, minlength=16)
    z0 = np.nonzero(lane_of_col < 0)[0]
    free = np.repeat(np.arange(16), np.maximum(TSUB - loads, 0))
    lane_of_col[z0] = free[:len(z0)]
    # fix any lane overflow: move lowest-degree cols to under-full lanes
    loads = np.bincount(lane_of_col, minlength=16)
    if loads.max() > TSUB:
        for j in np.nonzero(loads > TSUB)[0]:
            cj = np.nonzero(lane_of_col == j)[0]
            cj = cj[np.argsort(deg[cj])]  # move low-degree first
            surplus = cj[:loads[j] - TSUB]
            for c in surplus:
                tgt = int(np.argmin(loads))
                lane_of_col[c] = tgt
                loads[tgt] += 1; loads[j] -= 1
    off_of_col = np.zeros(ZT, np.int64)
    zperm = np.full((16, TSUB), -1, np.int64)
    for j in range(16):
        cj = np.nonzero(lane_of_col == j)[0]
        off_of_col[cj] = np.arange(len(cj))
        zperm[j, :len(cj)] = cj
    return (lane_of_col[col_g].astype(np.int64), off_of_col[col_g], zperm)

def _prep_static(rec_idx, rec_w):
    rows_g = rec_idx[:, 0].astype(np.int64)
    cols_g = rec_idx[:, 1].astype(np.int64)
    w_all = np.asarray(rec_w, np.float32)
    nc_id = rows_g // RPC

    # pass 1: per-NC per-edge fields + global class profile
    percore = []
    for m in range(NC_N):
        em = np.nonzero(nc_id == m)[0]
        r_loc = rows_g[em] - m * RPC
        lane, off, zperm = _balance_lanes(r_loc, cols_g[em])
        w = w_all[em]
        counts = np.bincount(r_loc * 16 + lane, minlength=RPC * 16).reshape(RPC, 16)
        Cr = np.maximum(counts.max(1).astype(np.int64), 1)
        order = np.argsort(-Cr, kind="stable")
        snake = np.tile(np.r_[np.arange(8), np.arange(7, -1, -1)],
                        RPC // 16 + 1)[:RPC]
        core_of_row = np.empty(RPC, np.int64)
        core_of_row[order] = snake
        percore.append((r_loc, lane, off, w, Cr, core_of_row, zperm))

    cmax = int(max(p[4].max() for p in percore))
    # global per-class row-count profile: max over all (NC, core) pairs
    prof = np.zeros(cmax + 1, np.int64)
    for (r_loc, lane, off, w, Cr, corow, zperm) in percore:
        for k in range(8):
            sel = corow == k
            h = np.bincount(Cr[sel], minlength=cmax + 1)
            prof = np.maximum(prof, h)
    classes = [c for c in range(cmax, 0, -1) if prof[c] > 0]

    # shared layout: position/slot for every (class, i) with CH-straddle padding
    plan = []  # (class, rowpos0, nrows, slot0) each within one chunk
    pos = 0; slot = 0
    cls_bounds = {}  # class -> (pos0, nrows)
    for c in classes:
        nr_ = int(prof[c]); done = 0
        cls_bounds[c] = (pos, nr_)
        while done < nr_:
            room = (-slot) % CH
            if room == 0:
                room = CH
            fit = min(nr_ - done, room // c)
            if fit == 0:
                slot += room
                continue
            plan.append((int(c), int(pos), int(fit), int(slot)))
            pos += fit; slot += fit * c; done += fit
    NROW_USED = pos
    NSLOT = int(-(-slot // CH) * CH)
    NCHUNK = NSLOT // CH
    assert NROW_USED <= NRQ, (NROW_USED, NRQ)
    # slot base per layout position
    slot_of_pos = np.zeros(NROW_USED, np.int64)
    for (c, p0, nr_, s0) in plan:
        slot_of_pos[p0:p0 + nr_] = s0 + np.arange(nr_) * c

    out = []
    for m in range(NC_N):
        (r_loc, lane, off, w, Cr, corow, zperm) = percore[m]
        # per-core: assign rows of class c to positions cls_bounds[c]
        rowpos = np.full((8, RPC), -1, np.int64)
        layout_rows = np.full((8, NRQ), -1, np.int64)
        for k in range(8):
            rk = np.nonzero(corow == k)[0]
            crk = Cr[rk]
            ordk = np.argsort(-crk, kind="stable")
            rk = rk[ordk]; crk = crk[ordk]
            # positions: within class c, i-th row -> cls_bounds[c][0] + i
            ranks = _rank_within_group(-crk)
            posk = np.array([cls_bounds[int(c)][0] for c in crk]) + ranks
            rowpos[k, rk] = posk
            layout_rows[k, posk] = rk

        ek = corow[r_loc]
        epos = rowpos[ek, r_loc]
        rank = _rank_within_group(r_loc * 16 + lane)
        slot_e = slot_of_pos[epos] + rank

        O = np.zeros((8, 16, NSLOT), np.int16)
        W = np.zeros((8, 16, NSLOT), np.float32)
        O[ek, lane, slot_e] = off.astype(np.int16)
        W[ek, lane, slot_e] = w

        idxs = np.zeros((NCHUNK, P, 16, CH // 16), np.int16)
        Wd = np.zeros((NCHUNK, P, CH), np.float32)
        Ov = O.reshape(8, 16, NCHUNK, CH)
        Wv = W.reshape(8, 16, NCHUNK, CH)
        for k in range(8):
            for j in range(16):
                idxs[:, 16 * k:16 * k + 16, j, :] = (
                    Ov[k, j].reshape(NCHUNK, CH // 16, 16).transpose(0, 2, 1))
                Wd[:, 16 * k + j, :] = Wv[k, j]

        rho = np.full(8 * NRQ, -1, np.int64)
        for k in range(8):
            rho[k * NRQ:(k + 1) * NRQ] = layout_rows[k]
        out.append(dict(idxs=idxs, Wd=Wd, rho=rho, zperm=zperm))
    return dict(percore=out, plan=plan, NCHUNK=NCHUNK, NSLOT=NSLOT)


# ---------------- device kernel ----------------
def _build_kernel(NCHUNK, plan):
    nc = bacc.Bacc("TRN2", target_bir_lowering=False, debug=False,
                   num_devices=NC_N)
    f32, i16 = mybir.dt.float32, mybir.dt.int16
    DI = lambda n, s, d=f32: nc.dram_tensor(n, s, d, kind="ExternalInput").ap()
    DO = lambda n, s, d=f32: nc.dram_tensor(n, s, d, kind="ExternalOutput").ap()

    z2d_d = DI("z2d", [16, TSUB])
    idxs_d = DI("idxs", [NCHUNK, P, 16, CH // 16], i16)
    wstr_d = DI("wstr", [NCHUNK, P, CH])
    ones_d = DI("onesb", [P, 8])
    syn_d = DI("syn", [5, P, F2])     # inputs, psc_rise, psc, syn_decay, psc_initial
    pscn_d = DI("pscn", [P, FN * R])  # psc in neuron order (padded)
    nrn_d = DI("nrn", [18, P, FN])
    nz_d = DO("nz", [P, FN]); outv_d = DO("outv", [P, FN])
    nv_d = DO("nv", [P, FN]); nr_d = DO("nr", [P, FN])
    na1_d = DO("na1", [P, FN]); na2_d = DO("na2", [P, FN])
    nprise_d = DO("nprise", [P, F2]); npsc_d = DO("npsc", [P, F2])

    AluOp = mybir.AluOpType
    ActF = mybir.ActivationFunctionType

    with tile.TileContext(nc) as tc, ExitStack() as octx:
        opool = octx.enter_context(tc.tile_pool(name="outer", bufs=1))
        partial = opool.tile([P, NRQ], f32)
        nc.vector.memset(partial[:], 0.0)

        with ExitStack() as gctx:
            gpool = gctx.enter_context(tc.tile_pool(name="g", bufs=1))
            dbl = gctx.enter_context(tc.tile_pool(name="gdbl", bufs=2))
            ztab = gpool.tile([P, TSUB], f32)
            for k in range(8):
                nc.sync.dma_start(ztab[16 * k:16 * (k + 1), :], z2d_d[:])
            for c in range(NCHUNK):
                T16 = dbl.tile([P, 16, CH], f32, tag="t16", name="T16")
                idxt = dbl.tile([P, 16 * (CH // 16)], i16, tag="idx")
                nc.scalar.dma_start(idxt[:],
                                 idxs_d[c].rearrange("p j s -> p (j s)"))
                wt = dbl.tile([P, CH], f32, tag="w")
                nc.scalar.dma_start(wt[:], wstr_d[c])
                nc.gpsimd.ap_gather(
                    out_ap=T16[:].rearrange("p j c -> p (j c)"), in_ap=ztab[:],
                    idxs_ap=idxt[:], channels=P, num_elems=TSUB, d=1,
                    num_idxs=16 * CH)
                V = dbl.tile([P, CH], f32, tag="v")
                for j in range(16):
                    nc.sync.dma_start(V[j:P:16, :], T16[j:P:16, j, :])
                nc.vector.tensor_mul(V[:], V[:], wt[:])
                for (cc, rowpos0, nrows, slot0) in plan:
                    if slot0 // CH != c:
                        continue
                    s0 = slot0 - c * CH
                    if cc == 1:
                        nc.vector.tensor_copy(
                            partial[:, rowpos0:rowpos0 + nrows],
                            V[:, s0:s0 + nrows])
                    else:
                        nc.vector.tensor_reduce(
                            partial[:, rowpos0:rowpos0 + nrows],
                            V[:, s0:s0 + nrows * cc].rearrange(
                                "p (n c) -> p n c", c=cc),
                            axis=mybir.AxisListType.X, op=AluOp.add)

        with ExitStack() as mctx:
            mpool = mctx.enter_context(tc.tile_pool(name="m", bufs=1))
            psum = mctx.enter_context(
                tc.tile_pool(name="ps", bufs=2, space="PSUM"))
            onesb = mpool.tile([P, 8], f32)
            nc.sync.dma_start(onesb[:], ones_d[:])
            irec8 = mpool.tile([8, NRQ], f32)
            t0 = 0
            while t0 < NRQ:
                csz = min(512, NRQ - t0)
                pt = psum.tile([8, 512], f32, space="PSUM", tag="pt")
                nc.tensor.matmul(out=pt[:, :csz], lhsT=onesb[:],
                                 rhs=partial[:, t0:t0 + csz],
                                 start=True, stop=True)
                nc.vector.tensor_copy(irec8[:, t0:t0 + csz], pt[:, :csz])
                t0 += csz
            # reshape [8, NRQ] -> [128, F2]: partition 16k+t <- irec8[k, t*F2:..]
            from concourse.ap import AP as RawAP
            i8ap = irec8[:]
            pitch = i8ap.ap[0][0]
            irec2 = mpool.tile([P, F2], f32)
            src = RawAP(tensor=i8ap.tensor, offset=i8ap.offset,
                        ap=[[pitch, 8], [F2, 16], [1, F2]])
            nc.sync.dma_start(irec2[:], src)

            # ---------------- phase 2 ----------------
            syn = [mpool.tile([P, F2], f32, tag=f"syn{i}", name=f"syn{i}") for i in range(5)]
            for i in range(5):
                nc.sync.dma_start(syn[i][:], syn_d[i])
            inp, prise, psc, sdec, pinit = syn
            pscn = mpool.tile([P, FN * R], f32)
            nc.sync.dma_start(pscn[:], pscn_d[:])
            nrn = [mpool.tile([P, FN], f32, tag=f"nrn{i}", name=f"nrn{i}") for i in range(18)]
            for i in range(18):
                nc.sync.dma_start(nrn[i][:], nrn_d[i])
            (v, r_, asc1, asc2, vth, el, vrst, g, dec, cf, tref,
             k0, k1, amp0, amp1, vscl, voff, pz) = nrn

            t1 = mpool.tile([P, F2], f32); t2 = mpool.tile([P, F2], f32)
            nc.vector.tensor_add(t1[:], irec2[:], inp[:])
            nc.vector.tensor_mul(t1[:], t1[:], pinit[:])
            nc.vector.tensor_mul(t2[:], sdec[:], prise[:])
            nc.vector.tensor_add(t1[:], t1[:], t2[:])
            nc.sync.dma_start(nprise_d[:], t1[:])
            nc.vector.tensor_add(t2[:], psc[:], prise[:])
            nc.vector.tensor_mul(t2[:], t2[:], sdec[:])
            nc.sync.dma_start(npsc_d[:], t2[:])

            a = mpool.tile([P, FN], f32); b = mpool.tile([P, FN], f32)
            c_ = mpool.tile([P, FN], f32); d_ = mpool.tile([P, FN], f32)
            cur = mpool.tile([P, FN], f32)
            nc.vector.tensor_reduce(
                cur[:], pscn[:].rearrange("p (n r) -> p n r", r=R),
                axis=mybir.AxisListType.X, op=AluOp.add)
            for (kk, ascx, amp, outd) in ((k0, asc1, amp0, na1_d),
                                          (k1, asc2, amp1, na2_d)):
                nc.scalar.activation(a[:], kk[:], ActF.Sigmoid)
                nc.scalar.activation(a[:], a[:], ActF.Exp, scale=-1.0)
                nc.vector.tensor_mul(a[:], a[:], ascx[:])
                nc.vector.tensor_mul(b[:], pz[:], amp[:])
                nc.vector.tensor_add(a[:], a[:], b[:])
                nc.sync.dma_start(outd[:], a[:])
            nc.vector.tensor_mul(a[:], pz[:], tref[:])
            nc.vector.tensor_add(a[:], a[:], r_[:])
            nc.vector.tensor_scalar_add(a[:], a[:], -DT)
            nc.vector.tensor_scalar_max(a[:], a[:], 0.0)
            nc.sync.dma_start(nr_d[:], a[:])          # a = new_r
            nc.vector.tensor_mul(b[:], g[:], el[:])
            nc.vector.tensor_add(b[:], b[:], cur[:])
            nc.vector.tensor_add(b[:], b[:], asc1[:])
            nc.vector.tensor_add(b[:], b[:], asc2[:])
            nc.vector.tensor_mul(b[:], b[:], cf[:])
            nc.vector.tensor_mul(c_[:], dec[:], v[:])
            nc.vector.tensor_add(b[:], b[:], c_[:])   # b = new_v (pre-reset)
            nc.vector.tensor_sub(c_[:], vth[:], el[:])
            nc.vector.reciprocal(c_[:], c_[:])
            nc.vector.tensor_sub(d_[:], b[:], vth[:])
            nc.vector.tensor_mul(d_[:], d_[:], c_[:])
            nc.vector.tensor_scalar(d_[:], d_[:], 0.0, None, op0=AluOp.is_gt)
            nc.vector.tensor_scalar(c_[:], a[:], 0.0, None, op0=AluOp.is_le)
            nc.vector.tensor_mul(d_[:], d_[:], c_[:])  # d_ = new_z
            nc.sync.dma_start(nz_d[:], d_[:])
            # blend: new_v = vrst*nz + b*(1-nz)
            nc.vector.tensor_scalar(t1[:, :FN], d_[:], -1.0, 1.0,
                                    op0=AluOp.mult, op1=AluOp.add)
            nc.vector.tensor_mul(b[:], b[:], t1[:, :FN])
            nc.vector.tensor_mul(c_[:], vrst[:], d_[:])
            nc.vector.tensor_add(c_[:], c_[:], b[:])
            nc.sync.dma_start(nv_d[:], c_[:])
            nc.vector.tensor_mul(c_[:], c_[:], vscl[:])
            nc.vector.tensor_add(c_[:], c_[:], voff[:])
            nc.sync.dma_start(outv_d[:], c_[:])
    nc.compile()
    return nc


# ---------------- runner ----------------
class _Runner:
    def __init__(self, nc, n_cores=NC_N):
        install_neuronx_cc_hook()
        self.nc = nc; self.n_cores = n_cores
        in_names, out_names, out_avals = [], [], []
        pname = nc.partition_id_tensor.name if nc.partition_id_tensor else None
        for alloc in nc.m.functions[0].allocations:
            if not isinstance(alloc, mybir.MemoryLocationSet):
                continue
            name = alloc.memorylocations[0].name
            if alloc.kind == "ExternalInput":
                if name != pname:
                    in_names.append(name)
            elif alloc.kind == "ExternalOutput":
                out_names.append(name)
                out_avals.append(jax.core.ShapedArray(
                    tuple(alloc.tensor_shape), mybir.dt.np(alloc.dtype)))
        self.in_names, self.out_names, self.out_avals = in_names, out_names, out_avals
        n_params = len(in_names); n_outs = len(out_avals)
        all_in = in_names + out_names + ([pname] if pname else [])
        donate = tuple(range(n_params, n_params + n_outs))
        use_pid = pname is not None

        def _body(*args):
            operands = list(args)
            if use_pid:
                operands.append(partition_id_tensor())
            return tuple(_bass_exec_p.bind(
                *operands, out_avals=tuple(out_avals), in_names=tuple(all_in),
                out_names=tuple(out_names), lowering_input_output_aliases=(),
                sim_require_finite=False, sim_require_nnan=False, nc=nc))

        devices = jax.devices()[:n_cores]
        mesh = Mesh(np.asarray(devices), ("core",))
        self.mesh = mesh
        self.fn = jax.jit(
            shard_map(_body, mesh=mesh,
                      in_specs=(PartitionSpec("core"),) * (n_params + n_outs),
                      out_specs=(PartitionSpec("core"),) * n_outs,
                      check_rep=False),
            donate_argnums=donate, keep_unused=True)
        self.n_params = n_params

    def run(self, in_maps):
        per_core = [[np.asarray(m[n]) for n in self.in_names] for m in in_maps]
        cat = [np.concatenate([per_core[c][i] for c in range(self.n_cores)], axis=0)
               for i in range(self.n_params)]
        zeros = [np.zeros((self.n_cores * a.shape[0], *a.shape[1:]), a.dtype)
                 for a in self.out_avals]
        outs = self.fn(*cat, *zeros)
        jax.block_until_ready(outs)
        return [
            {n: np.asarray(outs[i]).reshape(self.n_cores,
                                            *self.out_avals[i].shape)[c]
             for i, n in enumerate(self.out_names)}
            for c in range(self.n_cores)
        ]


_kernel_cache = {}


def kernel(inputs, z_buf, v, r, asc_1, asc_2, psc_rise, psc, rec_w, rec_idx,
           v_th, e_l, v_reset, g, decay, current_factor, t_ref, k, asc_amps,
           syn_decay, psc_initial, voltage_scale, voltage_offset):
    fp = _fingerprint(np.asarray(rec_idx))
    if fp not in _prep_cache:
        _prep_cache[fp] = _prep_static(np.asarray(rec_idx), np.asarray(rec_w))
    prep = _prep_cache[fp]
    NCHUNK, plan = prep["NCHUNK"], prep["plan"]

    kkey = (NCHUNK, tuple(plan))
    if kkey not in _kernel_cache:
        _kernel_cache[kkey] = _Runner(_build_kernel(NCHUNK, plan))
    runner = _kernel_cache[kkey]

    onesb = np.zeros((P, 8), np.float32)
    for pp in range(P):
        onesb[pp, pp // 16] = 1.0
    z_full = np.asarray(z_buf, np.float32).reshape(ZT)

    in_maps = []
    for m in range(NC_N):
        pr = prep["percore"][m]
        zperm = pr["zperm"]
        z2d = np.where(zperm >= 0, z_full[np.maximum(zperm, 0)], 0.0
                       ).astype(np.float32)
        rho = pr["rho"]
        valid = rho >= 0
        gsl = slice(m * RPC, (m + 1) * RPC)
        nsl = slice(m * NPC, (m + 1) * NPC)

        def synp(x):
            x = np.asarray(x, np.float32).reshape(-1)[gsl]
            out = np.zeros(8 * NRQ, np.float32)
            out[valid] = x[rho[valid]]
            return out.reshape(P, F2)

        syn = np.stack([synp(inputs), synp(psc_rise), synp(psc),
                        synp(syn_decay), synp(psc_initial)])
        pscn = np.zeros(NNP * R, np.float32)
        pscn[:RPC] = np.asarray(psc, np.float32).reshape(-1)[gsl]
        nfield = lambda x: np.pad(np.asarray(x, np.float32).reshape(-1)[nsl],
                                  (0, NNP - NPC)).reshape(P, FN)
        k_ = np.asarray(k, np.float32); aa = np.asarray(asc_amps, np.float32)
        nrn = np.stack([
            nfield(v), nfield(r), nfield(asc_1), nfield(asc_2), nfield(v_th),
            nfield(e_l), nfield(v_reset), nfield(g), nfield(decay),
            nfield(current_factor), nfield(t_ref), nfield(k_[:, 0]),
            nfield(k_[:, 1]), nfield(aa[:, 0]), nfield(aa[:, 1]),
            nfield(voltage_scale), nfield(voltage_offset),
            nfield(z_full[:N]),
        ])
        in_maps.append(dict(z2d=z2d, idxs=pr["idxs"], wstr=pr["Wd"],
                            onesb=onesb, syn=syn,
                            pscn=pscn.reshape(P, FN * R), nrn=nrn))

    global _last
    _last = (runner, in_maps)
    res = runner.run(in_maps)

    new_z = np.zeros(N, np.float32); out_v = np.zeros(N, np.float32)
    new_v = np.zeros(N, np.float32); new_r = np.zeros(N, np.float32)
    na1 = np.zeros(N, np.float32); na2 = np.zeros(N, np.float32)
    nprise = np.zeros(R * N, np.float32); npsc = np.zeros(R * N, np.float32)
    for m in range(NC_N):
        o = res[m]
        nsl = slice(m * NPC, (m + 1) * NPC)
        gsl = slice(m * RPC, (m + 1) * RPC)
        new_z[nsl] = o["nz"].reshape(-1)[:NPC]
        out_v[nsl] = o["outv"].reshape(-1)[:NPC]
        new_v[nsl] = o["nv"].reshape(-1)[:NPC]
        new_r[nsl] = o["nr"].reshape(-1)[:NPC]
        na1[nsl] = o["na1"].reshape(-1)[:NPC]
        na2[nsl] = o["na2"].reshape(-1)[:NPC]
        rho = prep["percore"][m]["rho"]; valid = rho >= 0
        t = np.zeros(RPC, np.float32)
        t[rho[valid]] = o["nprise"].reshape(-1)[valid]
        nprise[gsl] = t
        t = np.zeros(RPC, np.float32)
        t[rho[valid]] = o["npsc"].reshape(-1)[valid]
        npsc[gsl] = t

    zb = np.asarray(z_buf, np.float32).reshape(1, ZT)
    new_z_buf = np.concatenate([new_z.reshape(1, N), zb[:, :(D - 1) * N]],
                               axis=1)
    return (new_z.reshape(1, N), out_v.reshape(1, N), new_z_buf,
            new_v.reshape(1, N), new_r.reshape(1, N), na1.reshape(1, N),
            na2.reshape(1, N), nprise.reshape(1, R * N), npsc.reshape(1, R * N))


_last = None


def hw_time_s(iters=5):
    """Device-execution wall estimate: jitted fn with device-resident inputs."""
    runner, in_maps = _last
    per_core = [[np.asarray(m[n]) for n in runner.in_names] for m in in_maps]
    cat = [np.concatenate([per_core[c][i] for c in range(runner.n_cores)], axis=0)
           for i in range(runner.n_params)]
    from jax.sharding import NamedSharding
    sh = NamedSharding(runner.mesh, PartitionSpec("core"))
    ins = [jax.device_put(x, sh) for x in cat]
    jax.block_until_ready(ins)
    zeros = lambda: [np.zeros((runner.n_cores * a.shape[0], *a.shape[1:]), a.dtype)
                     for a in runner.out_avals]
    o = runner.fn(*ins, *zeros()); jax.block_until_ready(o)
    import time as _t
    ts = []
    for _ in range(iters):
        t0 = _t.perf_counter()
        o = runner.fn(*ins, *zeros()); jax.block_until_ready(o)
        ts.append(_t.perf_counter() - t0)
    return min(ts)
